# revision 1
# baseline (speedup 1.0000x reference)
"""Trainium2 Bass kernel for MiniCPMV ViT window-attention + 2x2 merger block.

Architecture (per reference):
  x[1,16384,1152] -> LN1 -> 2x2-window reorder -> QKV -> 4-token window attn
  (16 heads x 72) -> out-proj -> un-reorder + residual -> re-reorder ->
  [4096 windows x 4608] -> LN2 -> Linear(4608->17216) -> gelu(tanh) ->
  Linear(17216->1152) -> + mean-pool residual -> [1,4096,1152]

Key observation: the un-reorder after attention and the re-reorder before the
merger cancel, so everything stays in window order end-to-end and the output
is already in window (= merged token) order.

Sharding: pure data parallel over 8 cores; each core takes 2048 tokens
(512 windows, half of one image = 16 window-rows), weights replicated.
Token order within a core is (a, w): a = position-in-window (0..3),
w = window index (0..511), so per-a slices are contiguous.

On-chip layout is feature-major ([d on partitions, tokens on free axis]) the
whole way; the host pre-transposes x and all weights, so the kernel never
transposes activations (only the final 512x1152 output, via PE transpose).
"""

import numpy as np
import ml_dtypes

import concourse.bacc as bacc
import concourse.tile as tile
import concourse.bass as bass
from concourse import mybir
from concourse.bass_utils import run_bass_kernel_spmd

F32 = mybir.dt.float32
BF16 = mybir.dt.bfloat16
AF = mybir.ActivationFunctionType
ALU = mybir.AluOpType

# Problem constants (hardcoded per spec)
B, H, W, D, I, NH = 4, 64, 64, 1152, 4304, 16
T = B * H * W          # 16384 tokens
HD = D // NH           # 72 head dim
NCORES = 8
TS = T // NCORES       # 2048 tokens per core
NW = TS // 4           # 512 windows per core
DT = D // 128          # 9 feature tiles
JQ = 3 * D // 128      # 27 qkv output tiles
D4 = 4 * D             # 4608 merged feature dim
KT1 = D4 // 128        # 36 contraction tiles for w1
J1 = 4 * I             # 17216
J1P = 17280            # padded to 135*128
JT1 = J1P // 128       # 135
JBLK = 15              # w1 j-tiles per block
NBLK = JT1 // JBLK     # 9 blocks
CH = 8                 # stage-A chunks
WC = NW // CH          # 64 windows per chunk
TC = 4 * WC            # 256 tokens per chunk
EPS = 1e-6
SM_SCALE = 1.0 / np.sqrt(HD)


def build_program(debug=False):
    """Build the single-core SPMD program (same NEFF on all 8 cores)."""
    from contextlib import ExitStack

    nc = bacc.Bacc("TRN2", target_bir_lowering=False, num_devices=NCORES)

    # ---- DRAM parameters -------------------------------------------------
    def inp(name, shape, dtype):
        return nc.dram_tensor(name, shape, dtype, kind="ExternalInput").ap()

    xT_d = inp("xT", [DT, 128, TS], F32)            # feature-major x, cols (a,w)
    wqkv_d = inp("wqkv", [JQ, DT, 128, 128], BF16)  # [jt][dt] stationary tiles
    bqkv_d = inp("bqkv", [128, JQ], F32)
    wo_d = inp("wo", [DT, 128, D], F32 if False else BF16)  # [dtk][128][j]
    bo_d = inp("bo", [128, DT], F32)
    obd_d = inp("ones_bd", [DT, 128, NH], BF16)     # block-diag head masks
    obdT_d = inp("ones_bdT", [NH, DT, 128], BF16)
    w1_d = inp("w1t", [JT1, KT1, 128, 128], BF16)   # [jt][kt] stationary tiles
    b1_d = inp("b1", [128, JT1], F32)
    w2_d = inp("w2t", [DT, JT1, 128, 128], BF16)    # [dt][kt(j)] stationary tiles
    b2_d = inp("b2", [128, DT], F32)
    ident_d = inp("ident", [128, 128], F32)

    out_d = nc.dram_tensor("out", [NW, D], F32, kind="ExternalOutput").ap()

    # DRAM scratch for stage-A -> stage-B handoff
    skind = "ExternalOutput" if debug else "Internal"
    h2_d = nc.dram_tensor("h2_scratch", [DT, 128, 4, NW], BF16, kind=skind).ap()
    res_d = nc.dram_tensor("res_scratch", [DT, 128, NW], F32, kind=skind).ap()
    if debug:
        qkv_dump = nc.dram_tensor("qkv_dump", [JQ, 128, TS], BF16,
                                  kind="ExternalOutput").ap()
        y_dump = nc.dram_tensor("y_dump", [DT, 128, TS], F32,
                                kind="ExternalOutput").ap()
        attn_dump = nc.dram_tensor("attn_dump", [NH, 4, NW, 4], BF16,
                                   kind="ExternalOutput").ap()
        esb_dump = nc.dram_tensor("esb_dump", [NH, 4, NW, 4], F32,
                                  kind="ExternalOutput").ap()
        p_dump = nc.dram_tensor("p_dump", [128, 4, 4, NW], BF16,
                                kind="ExternalOutput").ap()

    with tile.TileContext(nc) as tc, ExitStack() as ctx:
        # ---- pools -------------------------------------------------------
        consts = ctx.enter_context(tc.tile_pool(name="consts", bufs=1))
        ps_main = ctx.enter_context(tc.tile_pool(name="ps_main", bufs=2, space="PSUM"))
        ps_st = ctx.enter_context(tc.tile_pool(name="ps_st", bufs=1, space="PSUM"))
        ps_sc = ctx.enter_context(tc.tile_pool(name="ps_sc", bufs=1, space="PSUM"))

        # ---- constants loaded once --------------------------------------
        wo_sb = consts.tile([128, DT, D], BF16)
        nc.sync.dma_start(wo_sb, wo_d.transpose([1, 0, 2]))
        bqkv_sb = consts.tile([128, JQ], F32)
        nc.sync.dma_start(bqkv_sb, bqkv_d)
        bo_sb = consts.tile([128, DT], F32)
        nc.sync.dma_start(bo_sb, bo_d)
        b1_sb = consts.tile([128, JT1], F32)
        nc.sync.dma_start(b1_sb, b1_d)
        b2_sb = consts.tile([128, DT], F32)
        nc.sync.dma_start(b2_sb, b2_d)
        obd_sb = consts.tile([128, DT, NH], BF16)
        nc.sync.dma_start(obd_sb, obd_d.transpose([1, 0, 2]))
        obdT_sb = consts.tile([NH, DT, 128], BF16)
        nc.sync.dma_start(obdT_sb, obdT_d)
        ident_sb = consts.tile([128, 128], F32)
        nc.sync.dma_start(ident_sb, ident_d)

        ones_col = consts.tile([128, 1], F32)
        nc.vector.memset(ones_col, 1.0)
        eps_t = consts.tile([1, 1], F32)
        nc.vector.memset(eps_t, EPS)

        # =================== Stage A: LN1 + attention =====================
        with ExitStack() as actx:
            xc_pool = actx.enter_context(tc.tile_pool(name="xc", bufs=2))
            wq_pool = actx.enter_context(tc.tile_pool(name="wq", bufs=4))
            h_pool = actx.enter_context(tc.tile_pool(name="h", bufs=2))
            qkv_pool = actx.enter_context(tc.tile_pool(name="qkv", bufs=2))
            p_pool = actx.enter_context(tc.tile_pool(name="p", bufs=3))
            sm_pool = actx.enter_context(tc.tile_pool(name="sm", bufs=2))
            av_pool = actx.enter_context(tc.tile_pool(name="av", bufs=2))
            o_pool = actx.enter_context(tc.tile_pool(name="o", bufs=2))
            y_pool = actx.enter_context(tc.tile_pool(name="y", bufs=2))
            st_pool = actx.enter_context(tc.tile_pool(name="st", bufs=2))
            h2c_pool = actx.enter_context(tc.tile_pool(name="h2c", bufs=2))

            for c in range(CH):
                w0 = c * WC

                # -- load x chunk: [128, dt, a, WC] fp32
                xc = xc_pool.tile([128, DT, 4, WC], F32)
                xsrc = xT_d.rearrange("t p (a w) -> t p a w", a=4)[:, :, :, w0:w0 + WC]
                for dt in range(DT):
                    nc.sync.dma_start(xc[:, dt], xsrc[dt])

                # -- LN1 stats: col sums of x and x^2 via ones-vector matmul
                stx = ps_st.tile([1, TC], F32, tag="stx")
                stq = ps_st.tile([1, TC], F32, tag="stq")
                for dt in range(DT):
                    xflat = xc[:, dt].rearrange("p a w -> p (a w)")
                    xsq = st_pool.tile([128, TC], F32, tag="xsq")
                    nc.vector.tensor_mul(xsq, xflat, xflat)
                    nc.tensor.matmul(stx, ones_col, xflat,
                                     start=(dt == 0), stop=(dt == DT - 1))
                    nc.tensor.matmul(stq, ones_col, xsq,
                                     start=(dt == 0), stop=(dt == DT - 1))

                mu = st_pool.tile([1, TC], F32, tag="mu")
                msq = st_pool.tile([1, TC], F32, tag="msq")
                nc.vector.tensor_scalar_mul(mu, stx, 1.0 / D)
                nc.vector.tensor_scalar_mul(msq, stq, 1.0 / D)
                var = st_pool.tile([1, TC], F32, tag="var")
                nc.vector.scalar_tensor_tensor(var, mu, -1.0, mu, ALU.mult, ALU.mult)  # -mu^2
                nc.vector.tensor_add(var, var, msq)
                rstd = st_pool.tile([1, TC], F32, tag="rstd")
                nc.scalar.activation(rstd, var, AF.Sqrt, bias=eps_t)
                nc.vector.reciprocal(rstd, rstd)
                nmu = st_pool.tile([1, TC], F32, tag="nmu")
                nc.vector.scalar_tensor_tensor(nmu, mu, -1.0, rstd, ALU.mult, ALU.mult)

                rstd_b = st_pool.tile([128, TC], F32, tag="rstd_b")
                nmu_b = st_pool.tile([128, TC], F32, tag="nmu_b")
                nc.gpsimd.partition_broadcast(rstd_b, rstd)
                nc.gpsimd.partition_broadcast(nmu_b, nmu)

                # -- normalize -> h bf16 [128, dt, TC]
                h = h_pool.tile([128, DT, TC], BF16)
                tmp = st_pool.tile([128, TC], F32, tag="normtmp")
                for dt in range(DT):
                    nc.vector.tensor_mul(tmp, xc[:, dt].rearrange("p a w -> p (a w)"), rstd_b)
                    nc.vector.tensor_add(h[:, dt], tmp, nmu_b)

                # -- QKV matmul: qkv bf16 [128, jt=27, TC]
                qkv = qkv_pool.tile([128, JQ, TC], BF16)
                for jt in range(JQ):
                    wq = wq_pool.tile([128, DT, 128], BF16)
                    nc.sync.dma_start(wq, wqkv_d[jt].transpose([1, 0, 2]))
                    mm = ps_main.tile([128, TC], F32, tag="mm")
                    for dt in range(DT):
                        nc.tensor.matmul(mm, wq[:, dt], h[:, dt],
                                         start=(dt == 0), stop=(dt == DT - 1))
                    nc.scalar.activation(qkv[:, jt], mm, AF.Identity,
                                         bias=bqkv_sb[:, jt:jt + 1])

                if debug:
                    for jt in range(JQ):
                        nc.sync.dma_start(
                            qkv_dump[jt].rearrange("p (a w) -> p a w", a=4)[:, :, w0:w0 + WC],
                            qkv[:, jt].rearrange("p (a w) -> p a w", a=4))

                # -- scores: p[dt] = q_sel * k_sel -> block-diag head reduce
                scs = []
                for qi in range(4):
                    sct = ps_sc.tile([NH, 4, WC], F32, tag=f"sc{qi}", name=f"sc{qi}_{c}")
                    scs.append(sct)
                for dt in range(DT):
                    q3 = qkv[:, dt].rearrange("p (a w) -> p a w", a=4)
                    k3 = qkv[:, DT + dt].rearrange("p (a w) -> p a w", a=4)
                    p_t = p_pool.tile([128, 4, 4, WC], BF16)
                    nc.vector.tensor_mul(
                        p_t,
                        q3.unsqueeze(2).to_broadcast([128, 4, 4, WC]),
                        k3.unsqueeze(1).to_broadcast([128, 4, 4, WC]),
                    )
                    if debug and dt == 0:
                        nc.sync.dma_start(p_dump[:, :, :, w0:w0 + WC], p_t)
                    for qi in range(4):
                        nc.tensor.matmul(scs[qi], obd_sb[:, dt], p_t[:, qi].rearrange("p a w -> p (a w)"),
                                         start=(dt == 0), stop=(dt == DT - 1))

                # -- softmax over ki (4): exp via ACT (fused scale), then norm
                esb = sm_pool.tile([NH, 4, WC, 4], F32, tag="esb")  # [h, qi, w, ki]
                for qi in range(4):
                    nc.scalar.activation(
                        esb[:, qi].transpose([0, 2, 1]),  # [h, ki, w] iter order
                        scs[qi],
                        AF.Exp, scale=float(SM_SCALE),
                    )
                if debug:
                    nc.sync.dma_start(esb_dump[:, :, w0:w0 + WC, :], esb)
                den = sm_pool.tile([NH, 4, WC], F32, tag="den")
                nc.vector.tensor_reduce(den, esb, axis=mybir.AxisListType.X, op=ALU.add)
                nc.vector.reciprocal(den, den)
                attn = sm_pool.tile([NH, 4, WC, 4], BF16, tag="attn")
                nc.vector.tensor_mul(
                    attn, esb,
                    den.unsqueeze(3).to_broadcast([NH, 4, WC, 4]),
                )

                # -- AV: expand attn to feature rows (PE), multiply by v, reduce ki
                o_bf = o_pool.tile([128, DT, 4 * WC], BF16)
                for dt in range(DT):
                    v3 = qkv[:, 2 * DT + dt].rearrange("p (a w) -> p a w", a=4)
                    for wv in range(2):
                        ex = ps_main.tile([128, 2, 4, WC], F32, tag="mm",
                                          name=f"ex{c}_{dt}_{wv}")
                        for qj in range(2):
                            nc.tensor.matmul(
                                ex[:, qj],
                                obdT_sb[:, dt],
                                attn[:, wv * 2 + qj].transpose([0, 2, 1]),
                                start=True, stop=True,
                            )
                        prod = av_pool.tile([128, 2, WC, 4], F32, tag="prod")
                        nc.vector.tensor_mul(
                            prod.transpose([0, 1, 3, 2]),      # iter (qj, ki, w)
                            ex,
                            v3.unsqueeze(1).to_broadcast([128, 2, 4, WC]),
                        )
                        with nc.allow_low_precision(reason="attn output bf16"):
                            nc.vector.tensor_reduce(
                                o_bf[:, dt].rearrange("p (a w) -> p a w", a=4)[:, wv * 2:wv * 2 + 2],
                                prod, axis=mybir.AxisListType.X, op=ALU.add,
                            )

                # -- out-projection + bias + residual -> y fp32
                y = y_pool.tile([128, DT, TC], F32)
                for dto in range(DT):
                    mm = ps_main.tile([128, TC], F32, tag="mm")
                    for dtk in range(DT):
                        nc.tensor.matmul(mm, wo_sb[:, dtk, dto * 128:(dto + 1) * 128],
                                         o_bf[:, dtk],
                                         start=(dtk == 0), stop=(dtk == DT - 1))
                    nc.vector.scalar_tensor_tensor(
                        y[:, dto], mm, bo_sb[:, dto:dto + 1],
                        xc[:, dto].rearrange("p a w -> p (a w)"),
                        ALU.add, ALU.add,
                    )

                if debug:
                    for dt in range(DT):
                        nc.sync.dma_start(
                            y_dump[dt].rearrange("p (a w) -> p a w", a=4)[:, :, w0:w0 + WC],
                            y[:, dt].rearrange("p (a w) -> p a w", a=4))
                    nc.sync.dma_start(attn_dump[:, :, w0:w0 + WC, :], attn)

                # -- LN2 stats over 4608 merged features (per window w)
                stx2 = ps_st.tile([1, TC], F32, tag="stx")
                stq2 = ps_st.tile([1, TC], F32, tag="stq")
                for dt in range(DT):
                    ysq = st_pool.tile([128, TC], F32, tag="xsq")
                    nc.vector.tensor_mul(ysq, y[:, dt], y[:, dt])
                    nc.tensor.matmul(stx2, ones_col, y[:, dt],
                                     start=(dt == 0), stop=(dt == DT - 1))
                    nc.tensor.matmul(stq2, ones_col, ysq,
                                     start=(dt == 0), stop=(dt == DT - 1))

                # fold the 4 a-positions: [1, (a w)] -> [1, w]
                s2a = st_pool.tile([1, WC], F32, tag="s2a")
                s2b = st_pool.tile([1, WC], F32, tag="s2b")
                nc.vector.tensor_reduce(
                    s2a, stx2.rearrange("p (a w) -> p w a", a=4),
                    axis=mybir.AxisListType.X, op=ALU.add)
                nc.vector.tensor_reduce(
                    s2b, stq2.rearrange("p (a w) -> p w a", a=4),
                    axis=mybir.AxisListType.X, op=ALU.add)
                mu2 = st_pool.tile([1, WC], F32, tag="mu2")
                msq2 = st_pool.tile([1, WC], F32, tag="msq2")
                nc.vector.tensor_scalar_mul(mu2, s2a, 1.0 / D4)
                nc.vector.tensor_scalar_mul(msq2, s2b, 1.0 / D4)
                var2 = st_pool.tile([1, WC], F32, tag="var2")
                nc.vector.scalar_tensor_tensor(var2, mu2, -1.0, mu2, ALU.mult, ALU.mult)
                nc.vector.tensor_add(var2, var2, msq2)
                rstd2 = st_pool.tile([1, WC], F32, tag="rstd2")
                nc.scalar.activation(rstd2, var2, AF.Sqrt, bias=eps_t)
                nc.vector.reciprocal(rstd2, rstd2)
                nmu2 = st_pool.tile([1, WC], F32, tag="nmu2")
                nc.vector.scalar_tensor_tensor(nmu2, mu2, -1.0, rstd2, ALU.mult, ALU.mult)

                rstd2_b = st_pool.tile([128, WC], F32, tag="rstd2_b")
                nmu2_b = st_pool.tile([128, WC], F32, tag="nmu2_b")
                nc.gpsimd.partition_broadcast(rstd2_b, rstd2)
                nc.gpsimd.partition_broadcast(nmu2_b, nmu2)

                # -- h2 = (y - mu2) * rstd2 (bf16), res = mean_a(y) / 4
                h2c = h2c_pool.tile([128, DT, 4, WC], BF16, tag="h2c")
                tmp2 = st_pool.tile([128, 4, WC], F32, tag="normtmp2")
                for dt in range(DT):
                    nc.vector.tensor_mul(
                        tmp2, y[:, dt].rearrange("p (a w) -> p a w", a=4),
                        rstd2_b.unsqueeze(1).to_broadcast([128, 4, WC]))
                    nc.vector.tensor_add(
                        h2c[:, dt], tmp2,
                        nmu2_b.unsqueeze(1).to_broadcast([128, 4, WC]))
                resc = h2c_pool.tile([128, DT, WC], F32, tag="resc")
                for dt in range(DT):
                    nc.vector.tensor_reduce(
                        resc[:, dt], y[:, dt].rearrange("p (a w) -> p w a", a=4),
                        axis=mybir.AxisListType.X, op=ALU.add)
                nc.vector.tensor_scalar_mul(resc, resc, 0.25)

                # -- spill chunk results to DRAM scratch
                for dt in range(DT):
                    nc.sync.dma_start(h2_d[dt, :, :, w0:w0 + WC], h2c[:, dt])
                nc.sync.dma_start(
                    res_d[:, :, w0:w0 + WC].transpose([1, 0, 2]), resc)

        # =================== Stage B: merger MLP ==========================
        with ExitStack() as bctx:
            h2_pool = bctx.enter_context(tc.tile_pool(name="h2f", bufs=1))
            res_pool = bctx.enter_context(tc.tile_pool(name="resf", bufs=1))
            acc_pool = bctx.enter_context(tc.tile_pool(name="acc", bufs=1))
            w1_pool = bctx.enter_context(tc.tile_pool(name="w1s", bufs=3))
            m2_pool = bctx.enter_context(tc.tile_pool(name="m2", bufs=2))
            w2_pool = bctx.enter_context(tc.tile_pool(name="w2s", bufs=3))
            fin_pool = bctx.enter_context(tc.tile_pool(name="fin", bufs=1))

            h2 = h2_pool.tile([128, DT, 4, NW], BF16)
            for dt in range(DT):
                nc.sync.dma_start(h2[:, dt], h2_d[dt])
            res = res_pool.tile([128, DT, NW], F32)
            nc.sync.dma_start(res, res_d.transpose([1, 0, 2]))
            acc = acc_pool.tile([128, DT, NW], F32)

            for blk in range(NBLK):
                m2 = m2_pool.tile([128, JBLK, NW], BF16)
                for j in range(JBLK):
                    jt = blk * JBLK + j
                    w1s = w1_pool.tile([128, KT1, 128], BF16)
                    nc.sync.dma_start(w1s, w1_d[jt].transpose([1, 0, 2]))
                    mm = ps_main.tile([128, NW], F32, tag="mm")
                    for kt in range(KT1):
                        a, dt = divmod(kt, DT)
                        nc.tensor.matmul(mm, w1s[:, kt], h2[:, dt, a],
                                         start=(kt == 0), stop=(kt == KT1 - 1))
                    nc.scalar.activation(m2[:, j], mm, AF.Gelu_apprx_tanh,
                                         bias=b1_sb[:, jt:jt + 1])

                for dt in range(DT):
                    w2s = w2_pool.tile([128, JBLK, 128], BF16)
                    nc.sync.dma_start(
                        w2s, w2_d[dt, blk * JBLK:(blk + 1) * JBLK].transpose([1, 0, 2]))
                    mm = ps_main.tile([128, NW], F32, tag="mm")
                    for j in range(JBLK):
                        nc.tensor.matmul(mm, w2s[:, j], m2[:, j],
                                         start=(j == 0), stop=(j == JBLK - 1))
                    if blk == 0:
                        nc.vector.tensor_add(acc[:, dt], mm, res[:, dt])
                    elif blk == NBLK - 1:
                        nc.vector.scalar_tensor_tensor(
                            acc[:, dt], mm, b2_sb[:, dt:dt + 1], acc[:, dt],
                            ALU.add, ALU.add)
                    else:
                        nc.vector.tensor_add(acc[:, dt], mm, acc[:, dt])

            # -- final transpose to token-major [512, 1152] and store
            fin = fin_pool.tile([128, 4, DT, 128], F32)
            for dt in range(DT):
                for mt in range(4):
                    tp = ps_main.tile([128, 128], F32, tag="mm")
                    nc.tensor.transpose(tp, acc[:, dt, mt * 128:(mt + 1) * 128],
                                        ident_sb)
                    nc.vector.tensor_copy(fin[:, mt, dt], tp)
            nc.sync.dma_start(
                out_d.rearrange("(mt p) (dt q) -> p mt dt q", p=128, q=128), fin)

    nc.compile()
    return nc


# ---------------------------------------------------------------------------
# Host side
# ---------------------------------------------------------------------------

_CACHED = {}


def make_runner(nc):
    """Build a reusable jitted SPMD executor for the finalized program.

    Mirrors concourse.bass2jax.run_bass_via_pjrt but caches the jitted
    callable so repeated kernel() calls (and benchmarking) don't recompile.
    Returns run(in_maps) -> list[dict] per core.
    """
    import jax
    from jax.sharding import Mesh, PartitionSpec
    from jax.experimental.shard_map import shard_map
    from concourse import mybir as _mybir
    from concourse.bass2jax import (
        install_neuronx_cc_hook, partition_id_tensor, _bass_exec_p)

    install_neuronx_cc_hook()
    partition_name = nc.partition_id_tensor.name if nc.partition_id_tensor else None

    in_names, out_names, out_avals, zero_shapes = [], [], [], []
    for alloc in nc.m.functions[0].allocations:
        if not isinstance(alloc, _mybir.MemoryLocationSet):
            continue
        name = alloc.memorylocations[0].name
        if alloc.kind == "ExternalInput":
            if name != partition_name:
                in_names.append(name)
        elif alloc.kind == "ExternalOutput":
            out_names.append(name)
            shape = tuple(alloc.tensor_shape)
            dtype = _mybir.dt.np(alloc.dtype)
            out_avals.append(jax.core.ShapedArray(shape, dtype))
            zero_shapes.append((shape, dtype))

    n_params = len(in_names)
    n_outs = len(out_avals)
    all_in_names = list(in_names) + list(out_names)
    if partition_name is not None:
        all_in_names.append(partition_name)
    donate = tuple(range(n_params, n_params + n_outs))

    def _body(*args):
        operands = list(args)
        if partition_name is not None:
            operands.append(partition_id_tensor())
        outs = _bass_exec_p.bind(
            *operands,
            out_avals=tuple(out_avals),
            in_names=tuple(all_in_names),
            out_names=tuple(out_names),
            lowering_input_output_aliases=(),
            sim_require_finite=True,
            sim_require_nnan=True,
            nc=nc,
        )
        return tuple(outs)

    devices = jax.devices()[:NCORES]
    mesh = Mesh(np.asarray(devices), ("core",))
    in_specs = (PartitionSpec("core"),) * (n_params + n_outs)
    out_specs = (PartitionSpec("core"),) * n_outs
    sharded = jax.jit(
        shard_map(_body, mesh=mesh, in_specs=in_specs, out_specs=out_specs,
                  check_rep=False),
        donate_argnums=donate, keep_unused=True)

    def make_zeros():
        return [np.zeros((NCORES * s[0], *s[1:]), d) for s, d in zero_shapes]

    def concat_inputs(in_maps):
        return [np.concatenate([np.asarray(in_maps[c][n]) for c in range(NCORES)],
                               axis=0)
                for n in in_names]

    def run(in_maps):
        out_arrs = sharded(*concat_inputs(in_maps), *make_zeros())
        return [
            {n: np.asarray(out_arrs[i]).reshape(NCORES, *out_avals[i].shape)[c]
             for i, n in enumerate(out_names)}
            for c in range(NCORES)
        ]

    run.sharded = sharded
    run.concat_inputs = concat_inputs
    run.make_zeros = make_zeros
    run.out_names = out_names
    run.out_avals = out_avals
    return run


def _prep_weights(ln1_g, ln1_b, w_qkv, b_qkv, w_o, b_o, pre_g, pre_b, w1, b1, w2, b2):
    bf = ml_dtypes.bfloat16
    f32 = np.float32

    ln1_g = np.asarray(ln1_g, f32)
    ln1_b = np.asarray(ln1_b, f32)
    w_qkv = np.asarray(w_qkv, f32)
    w1 = np.asarray(w1, f32)
    w2 = np.asarray(w2, f32)
    w_o = np.asarray(w_o, f32)
    pre_g = np.asarray(pre_g, f32)
    pre_b = np.asarray(pre_b, f32)

    wq = w_qkv * ln1_g[None, :]
    bq = w_qkv @ ln1_b + np.asarray(b_qkv, f32)
    wqkv_t = np.ascontiguousarray(
        wq.T.reshape(DT, 128, JQ, 128).transpose(2, 0, 1, 3)).astype(bf)
    bqkv_h = np.ascontiguousarray(bq.reshape(JQ, 128).T)

    wo_t = np.ascontiguousarray(w_o.T.reshape(DT, 128, D)).astype(bf)
    bo_h = np.ascontiguousarray(np.asarray(b_o, f32).reshape(DT, 128).T)

    w1g = w1 * pre_g[None, :]
    b1e = w1 @ pre_b + np.asarray(b1, f32)
    w1p = np.zeros((J1P, D4), f32)
    w1p[:J1] = w1g
    w1_t = np.ascontiguousarray(
        w1p.T.reshape(KT1, 128, JT1, 128).transpose(2, 0, 1, 3)).astype(bf)
    b1p = np.zeros((J1P,), f32)
    b1p[:J1] = b1e
    b1_h = np.ascontiguousarray(b1p.reshape(JT1, 128).T)

    w2p = np.zeros((J1P, D), f32)
    w2p[:J1] = w2.T
    w2_t = np.ascontiguousarray(
        w2p.reshape(JT1, 128, DT, 128).transpose(2, 0, 1, 3)).astype(bf)
    b2_h = np.ascontiguousarray(np.asarray(b2, f32).reshape(DT, 128).T)

    heads = (np.arange(D) // HD)
    obd = (heads[:, None] == np.arange(NH)[None, :]).astype(bf)      # [D, NH]
    obd_h = np.ascontiguousarray(obd.reshape(DT, 128, NH))
    obdT_h = np.ascontiguousarray(obd.T.reshape(NH, DT, 128))

    ident_h = np.eye(128, dtype=f32)

    return dict(
        wqkv=wqkv_t, bqkv=bqkv_h, wo=wo_t, bo=bo_h,
        ones_bd=obd_h, ones_bdT=obdT_h,
        w1t=w1_t, b1=b1_h, w2t=w2_t, b2=b2_h, ident=ident_h,
    )


def _shard_x(hidden_states):
    """Full x [1, T, D] -> per-core feature-major [DT, 128, TS] in (a, w) order."""
    x = np.asarray(hidden_states, np.float32)[0]          # [T, D]
    nh, nw = H // 2, W // 2
    xr = x.reshape(B, nh, 2, nw, 2, D)
    shards = []
    for c in range(NCORES):
        img, half = divmod(c, 2)
        sl = xr[img, half * 16:(half + 1) * 16]           # [16, 2, 32, 2, D]
        # (a=(r,cc), w=(i,j)) ordering
        sl = sl.transpose(1, 3, 0, 2, 4).reshape(TS, D)   # [(r c i j), D]
        xT = np.ascontiguousarray(sl.T).reshape(DT, 128, TS)
        shards.append(xT)
    return shards


def get_runner():
    if "runner" not in _CACHED:
        nc = build_program()
        _CACHED["runner"] = make_runner(nc)
    return _CACHED["runner"]


def make_in_maps(inputs):
    weights = _prep_weights(
        inputs["ln1_g"], inputs["ln1_b"], inputs["w_qkv"], inputs["b_qkv"],
        inputs["w_o"], inputs["b_o"], inputs["pre_g"], inputs["pre_b"],
        inputs["w1"], inputs["b1"], inputs["w2"], inputs["b2"])
    shards = _shard_x(inputs["hidden_states"])
    return [dict(weights, xT=shards[c]) for c in range(NCORES)]


def kernel(**inputs):
    run = get_runner()
    results = run(make_in_maps(inputs))
    out = np.concatenate([results[c]["out"] for c in range(NCORES)], axis=0)
    return out[None].astype(np.float32)



# revision 43
# speedup vs baseline: 1.0873x; 1.0873x over previous
"""Trainium2 Bass kernel for MiniCPMV ViT window-attention + 2x2 merger block.

Architecture (per reference):
  x[1,16384,1152] -> LN1 -> 2x2-window reorder -> QKV -> 4-token window attn
  (16 heads x 72) -> out-proj -> un-reorder + residual -> re-reorder ->
  [4096 windows x 4608] -> LN2 -> Linear(4608->17216) -> gelu(tanh) ->
  Linear(17216->1152) -> + mean-pool residual -> [1,4096,1152]

Key observation: the un-reorder after attention and the re-reorder before the
merger cancel, so everything stays in window order end-to-end and the output
is already in window (= merged token) order.

Sharding: pure data parallel over 8 cores; each core takes 2048 tokens
(512 windows, half of one image = 16 window-rows), weights replicated.
Token order within a core is (a, w): a = position-in-window (0..3),
w = window index (0..511), so per-a slices are contiguous.

v2 scheduling notes:
  - x is shipped bf16; QKV weights live resident in SBUF (loaded once).
  - Stage A is software-pipelined at emission level: PE order per chunk is
    qkv(c), ln1-stats(c+1), scores(c), AV(c), outproj(c), ln2-stats(c) so
    the DVE/ACT phases of chunk c overlap PE matmuls, and normalize of
    chunk c+1 happens during chunk c's attention tail.
  - h2 (LN2-normalized merger input) and res (mean-pool residual) stay
    resident in SBUF; no DRAM round trip between stages.
  - All weights are pre-transposed on the host so every DMA is contiguous.
"""

import numpy as np
import ml_dtypes

import concourse.bacc as bacc
import concourse.tile as tile
import concourse.bass as bass
from concourse import mybir
from concourse.bass_utils import run_bass_kernel_spmd

F32 = mybir.dt.float32
BF16 = mybir.dt.bfloat16
AF = mybir.ActivationFunctionType
ALU = mybir.AluOpType

# Problem constants (hardcoded per spec)
B, H, W, D, I, NH = 4, 64, 64, 1152, 4304, 16
T = B * H * W          # 16384 tokens
HD = D // NH           # 72 head dim
NCORES = 8
TS = T // NCORES       # 2048 tokens per core
NW = TS // 4           # 512 windows per core
DT = D // 128          # 9 feature tiles
JQ = 3 * D // 128      # 27 qkv output tiles
D4 = 4 * D             # 4608 merged feature dim
KT1 = D4 // 128        # 36 contraction tiles for w1
J1 = 4 * I             # 17216
J1P = 17280            # padded to 135*128
JT1 = J1P // 128       # 135
JBLK = 15              # w1 j-tiles per block
NBLK = JT1 // JBLK     # 9 blocks
CH = 8                 # stage-A chunks
WC = NW // CH          # 64 windows per chunk
TC = 4 * WC            # 256 tokens per chunk
EPS = 1e-6
SM_SCALE = 1.0 / np.sqrt(HD)


def build_program():
    """Build the single-core SPMD program (same NEFF on all 8 cores)."""
    from contextlib import ExitStack

    nc = bacc.Bacc("TRN2", target_bir_lowering=False, num_devices=NCORES)

    # ---- DRAM parameters (all laid out for contiguous DMA) ---------------
    def inp(name, shape, dtype):
        return nc.dram_tensor(name, shape, dtype, kind="ExternalInput").ap()

    xT_d = inp("xT", [DT, 128, TS], BF16)            # feature-major x, cols (a,w)
    wqkv_d = inp("wqkv", [128, JQ, DT, 128], BF16)   # [p][jt][dt][q]
    bqkv_d = inp("bqkv", [128, JQ], F32)
    wo_d = inp("wo", [128, DT, D], BF16)             # [p][dtk][j]
    bo_d = inp("bo", [128, DT], F32)
    obd_d = inp("ones_bd", [128, DT, NH], BF16)      # block-diag head masks
    obdT_d = inp("ones_bdT", [NH, DT, 128], BF16)    # head -> feature expand masks
    w1_d = inp("w1t", [JT1, 128, KT1, 128], BF16)    # [jt][p][kt][q]
    b1_d = inp("b1", [128, JT1], F32)
    w2_d = inp("w2t", [DT, 128, JT1, 128], BF16)     # [dt][p][jt][q]
    b2_d = inp("b2", [128, DT], F32)
    ident_d = inp("ident", [128, 128], F32)

    out_d = nc.dram_tensor("out", [NW, D], F32, kind="ExternalOutput").ap()

    with tile.TileContext(nc) as tc, ExitStack() as ctx:
        # ---- pools -------------------------------------------------------
        consts = ctx.enter_context(tc.tile_pool(name="consts", bufs=1))
        ps_main = ctx.enter_context(tc.tile_pool(name="ps_main", bufs=2, space="PSUM"))
        ps_sc = ctx.enter_context(tc.tile_pool(name="ps_sc", bufs=1, space="PSUM"))
        ps_ex = ctx.enter_context(tc.tile_pool(name="ps_ex", bufs=1, space="PSUM"))
        # h2 / res live across both stages
        h2_pool = ctx.enter_context(tc.tile_pool(name="h2f", bufs=1))
        res_pool = ctx.enter_context(tc.tile_pool(name="resf", bufs=1))

        # ---- constants (small ones DMA'd now; big ones after x chunk 0) --
        wo_sb = consts.tile([128, DT, D], BF16)
        bqkv_sb = consts.tile([128, JQ], F32)
        nc.sync.dma_start(bqkv_sb, bqkv_d)
        bo_sb = consts.tile([128, DT], F32)
        nc.sync.dma_start(bo_sb, bo_d)
        b1_sb = consts.tile([128, JT1], F32)
        b2_sb = consts.tile([128, DT], F32)
        nc.sync.dma_start(b2_sb, b2_d)
        obd_sb = consts.tile([128, DT, NH], BF16)
        nc.sync.dma_start(obd_sb, obd_d)
        obdT_sb = consts.tile([NH, DT, 128], BF16)
        nc.sync.dma_start(obdT_sb, obdT_d)
        ident_sb = consts.tile([128, 128], F32)

        ones_col = consts.tile([128, 1], BF16)
        nc.vector.memset(ones_col, 1.0)
        eps_t = consts.tile([1, 1], F32)
        nc.vector.memset(eps_t, EPS)

        h2 = h2_pool.tile([128, DT, 4, NW], BF16)    # LN2-normalized, resident
        res = res_pool.tile([128, DT, NW], BF16)     # mean-pool residual

        # =================== Stage A: LN1 + attention =====================
        with ExitStack() as actx:
            wq_pool = actx.enter_context(tc.tile_pool(name="wq", bufs=1))
            xb_pool = actx.enter_context(tc.tile_pool(name="xb", bufs=3))
            st_pool = actx.enter_context(tc.tile_pool(name="st", bufs=1))
            sq_pool = actx.enter_context(tc.tile_pool(name="sq", bufs=2))
            h_pool = actx.enter_context(tc.tile_pool(name="h", bufs=2))
            qkv_pool = actx.enter_context(tc.tile_pool(name="qkv", bufs=1))
            v_pool = actx.enter_context(tc.tile_pool(name="v", bufs=2))
            p_pool = actx.enter_context(tc.tile_pool(name="p", bufs=2))
            sm_pool = actx.enter_context(tc.tile_pool(name="sm", bufs=1))
            av_pool = actx.enter_context(tc.tile_pool(name="av", bufs=1))
            o_pool = actx.enter_context(tc.tile_pool(name="o", bufs=1))
            y_pool = actx.enter_context(tc.tile_pool(name="y", bufs=1))

            wq_sb = wq_pool.tile([128, JQ, DT, 128], BF16)

            S = [dict() for _ in range(CH)]

            def emit_dma_x(c):
                xb = xb_pool.tile([128, DT, 4, WC], BF16, tag="xb", name=f"xb{c}")
                xsrc = xT_d.rearrange("t p (a w) -> t p a w", a=4)[
                    :, :, :, c * WC:(c + 1) * WC]
                for dt in range(DT):
                    nc.sync.dma_start(xb[:, dt], xsrc[dt])
                S[c]["xb"] = xb

            def emit_stats(c):
                """LN1 stats: col sums of x and x^2 via ones-vector matmul."""
                xb = S[c]["xb"]
                stx = ps_main.tile([1, TC], F32, tag="mm", name=f"stx{c}")
                stq = ps_main.tile([1, TC], F32, tag="mm", name=f"stq{c}")
                for dt in range(DT):
                    nc.tensor.matmul(stx,
                                     ones_col,
                                     xb[:, dt].rearrange("p a w -> p (a w)"),
                                     start=(dt == 0), stop=(dt == DT - 1))
                for dt in range(DT):
                    xsq = sq_pool.tile([128, TC], BF16, tag="xsq")
                    nc.vector.tensor_mul(
                        xsq, xb[:, dt].rearrange("p a w -> p (a w)"),
                        xb[:, dt].rearrange("p a w -> p (a w)"))
                    nc.tensor.matmul(stq, ones_col, xsq,
                                     start=(dt == 0), stop=(dt == DT - 1))
                S[c]["stx"], S[c]["stq"] = stx, stq

            def bcast_bf16(val_f32, shape, tagp):
                """f32 [1, n] stats -> bf16 [128, n] broadcast tile."""
                n = shape[1]
                vb = st_pool.tile([1, n], BF16, tag=f"{tagp}_v")
                with nc.allow_low_precision(reason="LN scale bf16"):
                    nc.vector.tensor_copy(vb, val_f32)
                out = st_pool.tile([128, n], BF16, tag=f"{tagp}_b")
                nc.gpsimd.partition_broadcast(out, vb)
                return out

            def emit_finish_norm(c):
                xb = S[c]["xb"]
                mu = st_pool.tile([1, TC], F32, tag="mu")
                nc.vector.tensor_scalar_mul(mu, S[c]["stx"], 1.0 / D)
                var = st_pool.tile([1, TC], F32, tag="var")
                nc.vector.scalar_tensor_tensor(var, mu, -1.0, mu, ALU.mult, ALU.mult)
                nc.vector.scalar_tensor_tensor(var, S[c]["stq"], 1.0 / D, var,
                                               ALU.mult, ALU.add)
                rstd = st_pool.tile([1, TC], F32, tag="rstd")
                nc.scalar.activation(rstd, var, AF.Sqrt, bias=eps_t)
                nc.vector.reciprocal(rstd, rstd)
                nmu = mu  # in-place: mu := -mu * rstd
                nc.vector.scalar_tensor_tensor(nmu, mu, -1.0, rstd, ALU.mult, ALU.mult)

                rstd_b = bcast_bf16(rstd, [128, TC], "rstd")
                nmu_b = bcast_bf16(nmu, [128, TC], "nmu")

                h = h_pool.tile([128, DT, TC], BF16, tag="h", name=f"h{c}")
                for dt in range(DT):
                    tmp = st_pool.tile([128, TC], BF16, tag="normtmp")
                    nc.vector.tensor_mul(
                        tmp, xb[:, dt].rearrange("p a w -> p (a w)"), rstd_b)
                    nc.vector.tensor_add(h[:, dt], tmp, nmu_b)
                S[c]["h"] = h

            def emit_qkv_group(c, jt):
                h = S[c]["h"]
                if jt == 0:
                    S[c]["qk"] = qkv_pool.tile([128, 2 * DT, TC], BF16,
                                               tag="qk", name=f"qk{c}")
                    S[c]["v"] = v_pool.tile([128, DT, TC], BF16,
                                            tag="v", name=f"v{c}")
                mm = ps_main.tile([128, TC], F32, tag="mm")
                for dt in range(DT):
                    nc.tensor.matmul(mm, wq_sb[:, jt, dt], h[:, dt],
                                     start=(dt == 0), stop=(dt == DT - 1))
                dst = (S[c]["qk"][:, jt] if jt < 2 * DT
                       else S[c]["v"][:, jt - 2 * DT])
                nc.scalar.activation(dst, mm, AF.Identity,
                                     bias=bqkv_sb[:, jt:jt + 1])

            def emit_ptscores(c):
                qkv = S[c]["qk"]
                scs = [ps_sc.tile([NH, 4, WC], F32, tag=f"sc{qi}",
                                  name=f"sc{qi}_{c}") for qi in range(4)]
                for dt in range(DT):
                    q3 = qkv[:, dt].rearrange("p (a w) -> p a w", a=4)
                    k3 = qkv[:, DT + dt].rearrange("p (a w) -> p a w", a=4)
                    p_t = p_pool.tile([128, 4, 4, WC], BF16)
                    nc.vector.tensor_mul(
                        p_t,
                        q3.unsqueeze(2).to_broadcast([128, 4, 4, WC]),
                        k3.unsqueeze(1).to_broadcast([128, 4, 4, WC]),
                    )
                    for qi in range(4):
                        nc.tensor.matmul(
                            scs[qi], obd_sb[:, dt],
                            p_t[:, qi].rearrange("p a w -> p (a w)"),
                            start=(dt == 0), stop=(dt == DT - 1))
                S[c]["scs"] = scs

            def emit_softmax(c):
                scs = S[c]["scs"]
                # esb: [h, qi, w, ki] bf16, ki innermost
                esb = sm_pool.tile([NH, 4, WC, 4], BF16, tag="esb")
                for qi in range(4):
                    nc.scalar.activation(
                        esb[:, qi].rearrange("p w k -> p k w"), scs[qi],
                        AF.Exp, scale=float(SM_SCALE))
                den = sm_pool.tile([NH, 4, WC], BF16, tag="den")
                with nc.allow_low_precision(reason="softmax denom bf16"):
                    nc.vector.tensor_reduce(den, esb, axis=mybir.AxisListType.X,
                                            op=ALU.add)
                    nc.vector.reciprocal(den, den)
                attn = esb  # in-place normalize
                nc.vector.tensor_mul(
                    attn, esb, den.unsqueeze(3).to_broadcast([NH, 4, WC, 4]))
                S[c]["attn"] = attn

            def emit_av_dt(c, dt):
                attn = S[c]["attn"]
                if dt == 0:
                    S[c]["o_bf"] = o_pool.tile([128, DT, 4 * WC], BF16, tag="o",
                                               name=f"o{c}")
                o_bf = S[c]["o_bf"]
                # expand: ex[p, qi, ki, w] = attn[h(p), qi, w, ki]
                ex = ps_ex.tile([128, 4, 4, WC], F32, tag="ex",
                                name=f"ex{c}_{dt}")
                for qi in range(4):
                    nc.tensor.matmul(
                        ex[:, qi], obdT_sb[:, dt],
                        attn[:, qi].rearrange("p w k -> p k w"),
                        start=True, stop=True)
                exb = av_pool.tile([128, 4, 4, WC], BF16, tag="exb")
                nc.scalar.copy(exb, ex)
                v3 = S[c]["v"][:, dt].rearrange("p (k w) -> p k w", k=4)
                prod = av_pool.tile([128, 4, 4, WC], BF16, tag="prod")
                nc.vector.tensor_mul(
                    prod, exb,
                    v3.unsqueeze(1).to_broadcast([128, 4, 4, WC]))
                # reduce over ki as a packed add tree (cheaper than TensorReduce)
                nc.vector.tensor_add(prod[:, :, 0], prod[:, :, 0], prod[:, :, 1])
                nc.vector.tensor_add(prod[:, :, 2], prod[:, :, 2], prod[:, :, 3])
                nc.vector.tensor_add(
                    o_bf[:, dt].rearrange("p (a w) -> p a w", a=4),
                    prod[:, :, 0], prod[:, :, 2])

            def emit_outproj(c):
                o_bf, xb = S[c]["o_bf"], S[c]["xb"]
                yb = y_pool.tile([128, DT, TC], BF16, tag="y", name=f"y{c}")
                for dto in range(DT):
                    mm = ps_main.tile([128, TC], F32, tag="mm")
                    for dtk in range(DT):
                        nc.tensor.matmul(
                            mm, wo_sb[:, dtk, dto * 128:(dto + 1) * 128],
                            o_bf[:, dtk],
                            start=(dtk == 0), stop=(dtk == DT - 1))
                    t = st_pool.tile([128, TC], BF16, tag="ytmp")
                    nc.scalar.activation(t, mm, AF.Identity,
                                         bias=bo_sb[:, dto:dto + 1])
                    nc.vector.tensor_add(
                        yb[:, dto], t, xb[:, dto].rearrange("p a w -> p (a w)"))
                S[c]["yb"] = yb

            def emit_ln2(c):
                """LN2 stats over 4608 merged features; write h2/res resident."""
                yb = S[c]["yb"]
                w0 = c * WC
                st2x = ps_main.tile([1, TC], F32, tag="mm", name=f"st2x{c}")
                st2q = ps_main.tile([1, TC], F32, tag="mm", name=f"st2q{c}")
                for dt in range(DT):
                    nc.tensor.matmul(st2x, ones_col, yb[:, dt],
                                     start=(dt == 0), stop=(dt == DT - 1))
                for dt in range(DT):
                    ysq = sq_pool.tile([128, TC], BF16, tag="xsq")
                    nc.vector.tensor_mul(ysq, yb[:, dt], yb[:, dt])
                    nc.tensor.matmul(st2q, ones_col, ysq,
                                     start=(dt == 0), stop=(dt == DT - 1))

                # fold the 4 a-positions: [1, (a w)] -> [1, w]
                s2b = st_pool.tile([1, WC], F32, tag="s2b")
                mu2 = st_pool.tile([1, WC], F32, tag="mu2")
                nc.vector.tensor_reduce(
                    mu2, st2x.rearrange("p (a w) -> p w a", a=4),
                    axis=mybir.AxisListType.X, op=ALU.add)
                nc.vector.tensor_scalar_mul(mu2, mu2, 1.0 / D4)
                nc.vector.tensor_reduce(
                    s2b, st2q.rearrange("p (a w) -> p w a", a=4),
                    axis=mybir.AxisListType.X, op=ALU.add)
                var2 = st_pool.tile([1, WC], F32, tag="var2")
                nc.vector.scalar_tensor_tensor(var2, mu2, -1.0, mu2,
                                               ALU.mult, ALU.mult)
                nc.vector.scalar_tensor_tensor(var2, s2b, 1.0 / D4, var2,
                                               ALU.mult, ALU.add)
                rstd2 = st_pool.tile([1, WC], F32, tag="rstd2")
                nc.scalar.activation(rstd2, var2, AF.Sqrt, bias=eps_t)
                nc.vector.reciprocal(rstd2, rstd2)
                nmu2 = mu2  # in-place: mu2 := -mu2 * rstd2
                nc.vector.scalar_tensor_tensor(nmu2, mu2, -1.0, rstd2,
                                               ALU.mult, ALU.mult)

                rstd2_b = bcast_bf16(rstd2, [128, WC], "rstd2")
                nmu2_b = bcast_bf16(nmu2, [128, WC], "nmu2")

                # h2 = (y - mu2) * rstd2 (bf16); res = sum_a(y) (x0.25 in stage B)
                for dt in range(DT):
                    tmp2 = st_pool.tile([128, TC], BF16, tag="normtmp")
                    t2v = tmp2.rearrange("p (a w) -> p a w", a=4)
                    nc.vector.tensor_mul(
                        t2v, yb[:, dt].rearrange("p (a w) -> p a w", a=4),
                        rstd2_b.unsqueeze(1).to_broadcast([128, 4, WC]))
                    nc.vector.tensor_add(
                        h2[:, dt, :, w0:w0 + WC], t2v,
                        nmu2_b.unsqueeze(1).to_broadcast([128, 4, WC]))
                yv = yb.rearrange("p t (a w) -> p t a w", a=4)
                res_sl = res[:, :, w0:w0 + WC]
                nc.vector.tensor_add(res_sl, yv[:, :, 0], yv[:, :, 1])
                nc.vector.tensor_add(res_sl, res_sl, yv[:, :, 2])
                nc.vector.tensor_add(res_sl, res_sl, yv[:, :, 3])

            # ---- pipelined emission -------------------------------------
            # PE order per iteration c:
            #   [qkv(c) interleaved with av(c-1)] outproj(c-1) ln2(c-1)
            #   stats(c+1) scores(c)
            # so the DVE/ACT attention tail of c-1 hides under qkv(c)'s
            # matmuls and PE never waits on softmax/AV chains.
            emit_dma_x(0)
            emit_stats(0)
            emit_finish_norm(0)
            # x(0) is queued; now stream in the weights behind it
            for jt in range(JQ):
                nc.sync.dma_start(wq_sb[:, jt], wqkv_d[:, jt])
            nc.sync.dma_start(wo_sb, wo_d)
            nc.sync.dma_start(b1_sb, b1_d)
            nc.sync.dma_start(ident_sb, ident_d)
            for c in range(CH):
                if c + 1 < CH:
                    emit_dma_x(c + 1)
                for dt in range(DT):
                    emit_qkv_group(c, 3 * dt)
                    if c > 0:
                        emit_av_dt(c - 1, dt)
                    emit_qkv_group(c, 3 * dt + 1)
                    emit_qkv_group(c, 3 * dt + 2)
                if c > 0:
                    emit_outproj(c - 1)
                    emit_ln2(c - 1)
                if c + 1 < CH:
                    emit_stats(c + 1)
                    emit_finish_norm(c + 1)
                emit_ptscores(c)
                emit_softmax(c)
            for dt in range(DT):
                emit_av_dt(CH - 1, dt)
            emit_outproj(CH - 1)
            emit_ln2(CH - 1)

        # =================== Stage B: merger MLP ==========================
        with ExitStack() as bctx:
            acc_pool = bctx.enter_context(tc.tile_pool(name="acc", bufs=1))
            w1_pool = bctx.enter_context(tc.tile_pool(name="w1s", bufs=4))
            m2_pool = bctx.enter_context(tc.tile_pool(name="m2", bufs=2))
            w2_pool = bctx.enter_context(tc.tile_pool(name="w2s", bufs=3))
            fin_pool = bctx.enter_context(tc.tile_pool(name="fin", bufs=1))

            acc = acc_pool.tile([128, DT, NW], F32)
            fin = fin_pool.tile([128, 4, DT, 128], F32)

            for blk in range(NBLK):
                m2 = m2_pool.tile([128, JBLK, NW], BF16)
                for j in range(JBLK):
                    jt = blk * JBLK + j
                    w1s = w1_pool.tile([128, KT1, 128], BF16)
                    nc.sync.dma_start(w1s, w1_d[jt])
                    mm = ps_main.tile([128, NW], F32, tag="mm")
                    for kt in range(KT1):
                        a, dt = divmod(kt, DT)
                        nc.tensor.matmul(mm, w1s[:, kt], h2[:, dt, a],
                                         start=(kt == 0), stop=(kt == KT1 - 1))
                    nc.scalar.activation(m2[:, j], mm, AF.Gelu_apprx_tanh,
                                         bias=b1_sb[:, jt:jt + 1])

                for dt in range(DT):
                    w2s = w2_pool.tile([128, JBLK, 128], BF16)
                    nc.sync.dma_start(
                        w2s, w2_d[dt, :, blk * JBLK:(blk + 1) * JBLK])
                    mm = ps_main.tile([128, NW], F32, tag="mm")
                    for j in range(JBLK):
                        nc.tensor.matmul(mm, w2s[:, j], m2[:, j],
                                         start=(j == 0), stop=(j == JBLK - 1))
                    if blk == 0:
                        # res holds sum_a(y); x0.25 folds the mean-pool here
                        nc.vector.scalar_tensor_tensor(
                            acc[:, dt], res[:, dt], 0.25, mm, ALU.mult, ALU.add)
                    elif blk == NBLK - 1:
                        nc.vector.scalar_tensor_tensor(
                            acc[:, dt], mm, b2_sb[:, dt:dt + 1], acc[:, dt],
                            ALU.add, ALU.add)
                        # acc[:, dt] final: transpose to token-major now
                        for mt in range(4):
                            tp = ps_main.tile([128, 128], F32, tag="mm")
                            nc.tensor.transpose(
                                tp, acc[:, dt, mt * 128:(mt + 1) * 128],
                                ident_sb)
                            nc.vector.tensor_copy(fin[:, mt, dt], tp)
                    else:
                        nc.vector.tensor_add(acc[:, dt], mm, acc[:, dt])

            nc.sync.dma_start(
                out_d.rearrange("(mt p) (dt q) -> p mt dt q", p=128, q=128), fin)

    nc.compile()
    return nc


# ---------------------------------------------------------------------------
# Host side
# ---------------------------------------------------------------------------

_CACHED = {}


def make_runner(nc, chain=1):
    """Build a reusable jitted SPMD executor for the finalized program.

    Mirrors concourse.bass2jax.run_bass_via_pjrt but caches the jitted
    callable so repeated kernel() calls (and benchmarking) don't recompile.
    With chain=M the NEFF executes M times back-to-back inside one jit call
    (each call consumes the previous call's output buffer, so the chain
    cannot be elided) — used for slope-based timing.
    Returns run(in_maps) -> list[dict] per core.
    """
    import jax
    from jax.sharding import Mesh, PartitionSpec
    from jax.experimental.shard_map import shard_map
    from concourse import mybir as _mybir
    from concourse.bass2jax import (
        install_neuronx_cc_hook, partition_id_tensor, _bass_exec_p)

    install_neuronx_cc_hook()
    partition_name = nc.partition_id_tensor.name if nc.partition_id_tensor else None

    in_names, out_names, out_avals, zero_shapes = [], [], [], []
    for alloc in nc.m.functions[0].allocations:
        if not isinstance(alloc, _mybir.MemoryLocationSet):
            continue
        name = alloc.memorylocations[0].name
        if alloc.kind == "ExternalInput":
            if name != partition_name:
                in_names.append(name)
        elif alloc.kind == "ExternalOutput":
            out_names.append(name)
            shape = tuple(alloc.tensor_shape)
            dtype = _mybir.dt.np(alloc.dtype)
            out_avals.append(jax.core.ShapedArray(shape, dtype))
            zero_shapes.append((shape, dtype))

    n_params = len(in_names)
    n_outs = len(out_avals)
    all_in_names = list(in_names) + list(out_names)
    if partition_name is not None:
        all_in_names.append(partition_name)
    donate = tuple(range(n_params, n_params + n_outs))

    def _body(*args):
        params = list(args[:n_params])
        outs = list(args[n_params:])
        pid = [partition_id_tensor()] if partition_name is not None else []
        for _ in range(chain):
            outs = list(_bass_exec_p.bind(
                *(params + outs + pid),
                out_avals=tuple(out_avals),
                in_names=tuple(all_in_names),
                out_names=tuple(out_names),
                lowering_input_output_aliases=(),
                sim_require_finite=True,
                sim_require_nnan=True,
                nc=nc,
            ))
        return tuple(outs)

    import os
    if os.environ.get("BASS_SIM_CPU") == "1":
        devices = jax.devices("cpu")[:NCORES]
    else:
        devices = jax.devices()[:NCORES]
    mesh = Mesh(np.asarray(devices), ("core",))
    in_specs = (PartitionSpec("core"),) * (n_params + n_outs)
    out_specs = (PartitionSpec("core"),) * n_outs
    sharded = jax.jit(
        shard_map(_body, mesh=mesh, in_specs=in_specs, out_specs=out_specs,
                  check_rep=False),
        donate_argnums=donate, keep_unused=True)

    def make_zeros():
        return [np.zeros((NCORES * s[0], *s[1:]), d) for s, d in zero_shapes]

    def concat_inputs(in_maps):
        return [np.concatenate([np.asarray(in_maps[c][n]) for c in range(NCORES)],
                               axis=0)
                for n in in_names]

    def run(in_maps):
        out_arrs = sharded(*concat_inputs(in_maps), *make_zeros())
        return [
            {n: np.asarray(out_arrs[i]).reshape(NCORES, *out_avals[i].shape)[c]
             for i, n in enumerate(out_names)}
            for c in range(NCORES)
        ]

    run.sharded = sharded
    run.concat_inputs = concat_inputs
    run.make_zeros = make_zeros
    run.out_names = out_names
    run.out_avals = out_avals
    return run


def _prep_weights(ln1_g, ln1_b, w_qkv, b_qkv, w_o, b_o, pre_g, pre_b, w1, b1, w2, b2):
    bf = ml_dtypes.bfloat16
    f32 = np.float32

    ln1_g = np.asarray(ln1_g, f32)
    ln1_b = np.asarray(ln1_b, f32)
    w_qkv = np.asarray(w_qkv, f32)
    w1 = np.asarray(w1, f32)
    w2 = np.asarray(w2, f32)
    w_o = np.asarray(w_o, f32)
    pre_g = np.asarray(pre_g, f32)
    pre_b = np.asarray(pre_b, f32)

    wq = w_qkv * ln1_g[None, :]
    bq = w_qkv @ ln1_b + np.asarray(b_qkv, f32)
    # [p, jt, dt, q] = wq[jt*128+q, dt*128+p]
    wqkv_t = np.ascontiguousarray(
        wq.T.reshape(DT, 128, JQ, 128).transpose(1, 2, 0, 3)).astype(bf)
    bqkv_h = np.ascontiguousarray(bq.reshape(JQ, 128).T)

    # [p, dtk, j] = w_o[j, dtk*128+p]
    wo_t = np.ascontiguousarray(
        w_o.T.reshape(DT, 128, D).transpose(1, 0, 2)).astype(bf)
    bo_h = np.ascontiguousarray(np.asarray(b_o, f32).reshape(DT, 128).T)

    w1g = w1 * pre_g[None, :]
    b1e = w1 @ pre_b + np.asarray(b1, f32)
    w1p = np.zeros((J1P, D4), f32)
    w1p[:J1] = w1g
    # [jt, p, kt, q] = w1p[jt*128+q, kt*128+p]
    w1_t = np.ascontiguousarray(
        w1p.T.reshape(KT1, 128, JT1, 128).transpose(2, 1, 0, 3)).astype(bf)
    b1p = np.zeros((J1P,), f32)
    b1p[:J1] = b1e
    b1_h = np.ascontiguousarray(b1p.reshape(JT1, 128).T)

    w2p = np.zeros((J1P, D), f32)
    w2p[:J1] = w2.T
    # [dt, p, jt, q] = w2p[jt*128+p, dt*128+q]
    w2_t = np.ascontiguousarray(
        w2p.reshape(JT1, 128, DT, 128).transpose(2, 1, 0, 3)).astype(bf)
    b2_h = np.ascontiguousarray(np.asarray(b2, f32).reshape(DT, 128).T)

    heads = (np.arange(D) // HD)
    obd = (heads[:, None] == np.arange(NH)[None, :]).astype(bf)      # [D, NH]
    obd_h = np.ascontiguousarray(obd.reshape(DT, 128, NH).transpose(1, 0, 2))
    obdT_h = np.ascontiguousarray(obd.T.reshape(NH, DT, 128))

    ident_h = np.eye(128, dtype=f32)

    return dict(
        wqkv=wqkv_t, bqkv=bqkv_h, wo=wo_t, bo=bo_h,
        ones_bd=obd_h, ones_bdT=obdT_h,
        w1t=w1_t, b1=b1_h, w2t=w2_t, b2=b2_h, ident=ident_h,
    )


def _shard_x(hidden_states):
    """Full x [1, T, D] -> per-core feature-major bf16 [DT, 128, TS] (a, w)."""
    x = np.asarray(hidden_states, np.float32)[0]          # [T, D]
    nh, nw = H // 2, W // 2
    xr = x.reshape(B, nh, 2, nw, 2, D)
    shards = []
    for c in range(NCORES):
        img, half = divmod(c, 2)
        sl = xr[img, half * 16:(half + 1) * 16]           # [16, 2, 32, 2, D]
        # (a=(r,cc), w=(i,j)) ordering
        sl = sl.transpose(1, 3, 0, 2, 4).reshape(TS, D)   # [(r c i j), D]
        xT = np.ascontiguousarray(sl.T).reshape(DT, 128, TS)
        shards.append(xT.astype(ml_dtypes.bfloat16))
    return shards


def get_runner():
    if "runner" not in _CACHED:
        nc = build_program()
        _CACHED["nc"] = nc
        _CACHED["runner"] = make_runner(nc)
    return _CACHED["runner"]


def make_in_maps(inputs):
    weights = _prep_weights(
        inputs["ln1_g"], inputs["ln1_b"], inputs["w_qkv"], inputs["b_qkv"],
        inputs["w_o"], inputs["b_o"], inputs["pre_g"], inputs["pre_b"],
        inputs["w1"], inputs["b1"], inputs["w2"], inputs["b2"])
    shards = _shard_x(inputs["hidden_states"])
    return [dict(weights, xT=shards[c]) for c in range(NCORES)]


def kernel(**inputs):
    run = get_runner()
    results = run(make_in_maps(inputs))
    out = np.concatenate([results[c]["out"] for c in range(NCORES)], axis=0)
    return out[None].astype(np.float32)


# revision 46
# speedup vs baseline: 1.1279x; 1.0373x over previous
"""Trainium2 Bass kernel for MiniCPMV ViT window-attention + 2x2 merger block.

Architecture (per reference):
  x[1,16384,1152] -> LN1 -> 2x2-window reorder -> QKV -> 4-token window attn
  (16 heads x 72) -> out-proj -> un-reorder + residual -> re-reorder ->
  [4096 windows x 4608] -> LN2 -> Linear(4608->17216) -> gelu(tanh) ->
  Linear(17216->1152) -> + mean-pool residual -> [1,4096,1152]

Key observation: the un-reorder after attention and the re-reorder before the
merger cancel, so everything stays in window order end-to-end and the output
is already in window (= merged token) order.

Sharding: pure data parallel over 8 cores; each core takes 2048 tokens
(512 windows, half of one image = 16 window-rows), weights replicated.
Token order within a core is (a, w): a = position-in-window (0..3),
w = window index (0..511), so per-a slices are contiguous.

v2 scheduling notes:
  - x is shipped bf16; QKV weights live resident in SBUF (loaded once).
  - Stage A is software-pipelined at emission level: PE order per chunk is
    qkv(c), ln1-stats(c+1), scores(c), AV(c), outproj(c), ln2-stats(c) so
    the DVE/ACT phases of chunk c overlap PE matmuls, and normalize of
    chunk c+1 happens during chunk c's attention tail.
  - h2 (LN2-normalized merger input) and res (mean-pool residual) stay
    resident in SBUF; no DRAM round trip between stages.
  - All weights are pre-transposed on the host so every DMA is contiguous.
"""

import numpy as np
import ml_dtypes

import concourse.bacc as bacc
import concourse.tile as tile
import concourse.bass as bass
from concourse import mybir
from concourse.bass_utils import run_bass_kernel_spmd

F32 = mybir.dt.float32
BF16 = mybir.dt.bfloat16
AF = mybir.ActivationFunctionType
ALU = mybir.AluOpType

# Problem constants (hardcoded per spec)
B, H, W, D, I, NH = 4, 64, 64, 1152, 4304, 16
T = B * H * W          # 16384 tokens
HD = D // NH           # 72 head dim
NCORES = 8
TS = T // NCORES       # 2048 tokens per core
NW = TS // 4           # 512 windows per core
DT = D // 128          # 9 feature tiles
JQ = 3 * D // 128      # 27 qkv output tiles
D4 = 4 * D             # 4608 merged feature dim
KT1 = D4 // 128        # 36 contraction tiles for w1
J1 = 4 * I             # 17216
J1P = 17280            # padded to 135*128
JT1 = J1P // 128       # 135
JBLK = 15              # w1 j-tiles per block
NBLK = JT1 // JBLK     # 9 blocks
CH = 8                 # stage-A chunks
WC = NW // CH          # 64 windows per chunk
TC = 4 * WC            # 256 tokens per chunk
EPS = 1e-6
SM_SCALE = 1.0 / np.sqrt(HD)


def build_program():
    """Build the single-core SPMD program (same NEFF on all 8 cores)."""
    from contextlib import ExitStack

    nc = bacc.Bacc("TRN2", target_bir_lowering=False, num_devices=NCORES)

    # ---- DRAM parameters (all laid out for contiguous DMA) ---------------
    def inp(name, shape, dtype):
        return nc.dram_tensor(name, shape, dtype, kind="ExternalInput").ap()

    xT_d = inp("xT", [DT, 128, TS], BF16)            # feature-major x, cols (a,w)
    wqkv_d = inp("wqkv", [128, JQ, DT, 128], BF16)   # [p][jt][dt][q]
    bqkv_d = inp("bqkv", [128, JQ], F32)
    wo_d = inp("wo", [128, DT, D], BF16)             # [p][dtk][j]
    bo_d = inp("bo", [128, DT], F32)
    obd_d = inp("ones_bd", [128, DT, NH], BF16)      # block-diag head masks
    obdT_d = inp("ones_bdT", [NH, DT, 128], BF16)    # head -> feature expand masks
    w1_d = inp("w1t", [JT1, 128, KT1, 128], BF16)    # [jt][p][kt][q]
    b1_d = inp("b1", [128, JT1], F32)
    w2_d = inp("w2t", [DT, 128, JT1, 128], BF16)     # [dt][p][jt][q]
    b2_d = inp("b2", [128, DT], F32)
    ident_d = inp("ident", [128, 128], F32)

    out_d = nc.dram_tensor("out", [NW, D], F32, kind="ExternalOutput").ap()

    with tile.TileContext(nc) as tc, ExitStack() as ctx:
        # ---- pools -------------------------------------------------------
        consts = ctx.enter_context(tc.tile_pool(name="consts", bufs=1))
        ps_main = ctx.enter_context(tc.tile_pool(name="ps_main", bufs=2, space="PSUM"))
        ps_sc = ctx.enter_context(tc.tile_pool(name="ps_sc", bufs=1, space="PSUM"))
        ps_ex = ctx.enter_context(tc.tile_pool(name="ps_ex", bufs=1, space="PSUM"))
        # h2 / res live across both stages
        h2_pool = ctx.enter_context(tc.tile_pool(name="h2f", bufs=1))
        res_pool = ctx.enter_context(tc.tile_pool(name="resf", bufs=1))
        # first w1 tile prefetched during stage A so stage B starts instantly
        w1f_pool = ctx.enter_context(tc.tile_pool(name="w1f", bufs=1))

        # ---- constants (small ones DMA'd now; big ones after x chunk 0) --
        wo_sb = consts.tile([128, DT, D], BF16)
        bqkv_sb = consts.tile([128, JQ], F32)
        nc.sync.dma_start(bqkv_sb, bqkv_d)
        bo_sb = consts.tile([128, DT], F32)
        nc.sync.dma_start(bo_sb, bo_d)
        b1_sb = consts.tile([128, JT1], F32)
        b2_sb = consts.tile([128, DT], F32)
        nc.sync.dma_start(b2_sb, b2_d)
        obd_sb = consts.tile([128, DT, NH], BF16)
        nc.sync.dma_start(obd_sb, obd_d)
        obdT_sb = consts.tile([NH, DT, 128], BF16)
        nc.sync.dma_start(obdT_sb, obdT_d)
        ident_sb = consts.tile([128, 128], F32)

        ones_col = consts.tile([128, 1], BF16)
        nc.vector.memset(ones_col, 1.0)
        eps_t = consts.tile([1, 1], F32)
        nc.vector.memset(eps_t, EPS)

        h2 = h2_pool.tile([128, DT, 4, NW], BF16)    # LN2-normalized, resident
        res = res_pool.tile([128, DT, NW], BF16)     # mean-pool residual
        w1f_sb = w1f_pool.tile([128, KT1, 128], BF16)

        # =================== Stage A: LN1 + attention =====================
        with ExitStack() as actx:
            wq_pool = actx.enter_context(tc.tile_pool(name="wq", bufs=1))
            xb_pool = actx.enter_context(tc.tile_pool(name="xb", bufs=3))
            st_pool = actx.enter_context(tc.tile_pool(name="st", bufs=1))
            sq_pool = actx.enter_context(tc.tile_pool(name="sq", bufs=2))
            h_pool = actx.enter_context(tc.tile_pool(name="h", bufs=2))
            qkv_pool = actx.enter_context(tc.tile_pool(name="qkv", bufs=1))
            v_pool = actx.enter_context(tc.tile_pool(name="v", bufs=2))
            p_pool = actx.enter_context(tc.tile_pool(name="p", bufs=2))
            sm_pool = actx.enter_context(tc.tile_pool(name="sm", bufs=1))
            av_pool = actx.enter_context(tc.tile_pool(name="av", bufs=1))
            o_pool = actx.enter_context(tc.tile_pool(name="o", bufs=1))
            y_pool = actx.enter_context(tc.tile_pool(name="y", bufs=1))

            wq_sb = wq_pool.tile([128, JQ, DT, 128], BF16)

            S = [dict() for _ in range(CH)]

            def emit_dma_x(c):
                xb = xb_pool.tile([128, DT, 4, WC], BF16, tag="xb", name=f"xb{c}")
                xsrc = xT_d.rearrange("t p (a w) -> t p a w", a=4)[
                    :, :, :, c * WC:(c + 1) * WC]
                for dt in range(DT):
                    nc.sync.dma_start(xb[:, dt], xsrc[dt])
                S[c]["xb"] = xb

            def emit_stats(c):
                """LN1 stats: col sums of x and x^2 via ones-vector matmul."""
                xb = S[c]["xb"]
                stx = ps_main.tile([1, TC], F32, tag="mm", name=f"stx{c}")
                stq = ps_main.tile([1, TC], F32, tag="mm", name=f"stq{c}")
                for dt in range(DT):
                    nc.tensor.matmul(stx,
                                     ones_col,
                                     xb[:, dt].rearrange("p a w -> p (a w)"),
                                     start=(dt == 0), stop=(dt == DT - 1))
                for dt in range(DT):
                    xsq = sq_pool.tile([128, TC], BF16, tag="xsq")
                    nc.vector.tensor_mul(
                        xsq, xb[:, dt].rearrange("p a w -> p (a w)"),
                        xb[:, dt].rearrange("p a w -> p (a w)"))
                    nc.tensor.matmul(stq, ones_col, xsq,
                                     start=(dt == 0), stop=(dt == DT - 1))
                S[c]["stx"], S[c]["stq"] = stx, stq

            def bcast_bf16(val_f32, shape, tagp):
                """f32 [1, n] stats -> bf16 [128, n] broadcast tile."""
                n = shape[1]
                vb = st_pool.tile([1, n], BF16, tag=f"{tagp}_v")
                with nc.allow_low_precision(reason="LN scale bf16"):
                    nc.vector.tensor_copy(vb, val_f32)
                out = st_pool.tile([128, n], BF16, tag=f"{tagp}_b")
                nc.gpsimd.partition_broadcast(out, vb)
                return out

            def emit_finish_norm(c):
                xb = S[c]["xb"]
                mu = st_pool.tile([1, TC], F32, tag="mu")
                nc.vector.tensor_scalar_mul(mu, S[c]["stx"], 1.0 / D)
                var = st_pool.tile([1, TC], F32, tag="var")
                nc.vector.scalar_tensor_tensor(var, mu, -1.0, mu, ALU.mult, ALU.mult)
                nc.vector.scalar_tensor_tensor(var, S[c]["stq"], 1.0 / D, var,
                                               ALU.mult, ALU.add)
                rstd = st_pool.tile([1, TC], F32, tag="rstd")
                nc.scalar.activation(rstd, var, AF.Sqrt, bias=eps_t)
                nc.vector.reciprocal(rstd, rstd)
                nmu = mu  # in-place: mu := -mu * rstd
                nc.vector.scalar_tensor_tensor(nmu, mu, -1.0, rstd, ALU.mult, ALU.mult)

                rstd_b = bcast_bf16(rstd, [128, TC], "rstd")
                nmu_b = bcast_bf16(nmu, [128, TC], "nmu")

                h = h_pool.tile([128, DT, TC], BF16, tag="h", name=f"h{c}")
                for dt in range(DT):
                    tmp = st_pool.tile([128, TC], BF16, tag="normtmp")
                    nc.vector.tensor_mul(
                        tmp, xb[:, dt].rearrange("p a w -> p (a w)"), rstd_b)
                    nc.vector.tensor_add(h[:, dt], tmp, nmu_b)
                S[c]["h"] = h

            def emit_qkv_group(c, jt):
                h = S[c]["h"]
                if jt == 0:
                    S[c]["qk"] = qkv_pool.tile([128, 2 * DT, TC], BF16,
                                               tag="qk", name=f"qk{c}")
                    S[c]["v"] = v_pool.tile([128, DT, TC], BF16,
                                            tag="v", name=f"v{c}")
                mm = ps_main.tile([128, TC], F32, tag="mm")
                for dt in range(DT):
                    nc.tensor.matmul(mm, wq_sb[:, jt, dt], h[:, dt],
                                     start=(dt == 0), stop=(dt == DT - 1))
                dst = (S[c]["qk"][:, jt] if jt < 2 * DT
                       else S[c]["v"][:, jt - 2 * DT])
                nc.scalar.activation(dst, mm, AF.Identity,
                                     bias=bqkv_sb[:, jt:jt + 1])

            def emit_ptscores(c):
                qkv = S[c]["qk"]
                scs = [ps_sc.tile([NH, 4, WC], F32, tag=f"sc{qi}",
                                  name=f"sc{qi}_{c}") for qi in range(4)]
                for dt in range(DT):
                    q3 = qkv[:, dt].rearrange("p (a w) -> p a w", a=4)
                    k3 = qkv[:, DT + dt].rearrange("p (a w) -> p a w", a=4)
                    p_t = p_pool.tile([128, 4, 4, WC], BF16)
                    nc.vector.tensor_mul(
                        p_t,
                        q3.unsqueeze(2).to_broadcast([128, 4, 4, WC]),
                        k3.unsqueeze(1).to_broadcast([128, 4, 4, WC]),
                    )
                    for qi in range(4):
                        nc.tensor.matmul(
                            scs[qi], obd_sb[:, dt],
                            p_t[:, qi].rearrange("p a w -> p (a w)"),
                            start=(dt == 0), stop=(dt == DT - 1))
                S[c]["scs"] = scs

            def emit_softmax(c):
                scs = S[c]["scs"]
                # esb: [h, qi, w, ki] bf16, ki innermost
                esb = sm_pool.tile([NH, 4, WC, 4], BF16, tag="esb")
                for qi in range(4):
                    nc.scalar.activation(
                        esb[:, qi].rearrange("p w k -> p k w"), scs[qi],
                        AF.Exp, scale=float(SM_SCALE))
                den = sm_pool.tile([NH, 4, WC], BF16, tag="den")
                with nc.allow_low_precision(reason="softmax denom bf16"):
                    nc.vector.tensor_reduce(den, esb, axis=mybir.AxisListType.X,
                                            op=ALU.add)
                    nc.vector.reciprocal(den, den)
                attn = esb  # in-place normalize
                nc.vector.tensor_mul(
                    attn, esb, den.unsqueeze(3).to_broadcast([NH, 4, WC, 4]))
                S[c]["attn"] = attn

            def emit_av_dt(c, dt):
                attn = S[c]["attn"]
                if dt == 0:
                    S[c]["o_bf"] = o_pool.tile([128, DT, 4 * WC], BF16, tag="o",
                                               name=f"o{c}")
                o_bf = S[c]["o_bf"]
                # expand: ex[p, qi, ki, w] = attn[h(p), qi, w, ki]
                ex = ps_ex.tile([128, 4, 4, WC], F32, tag="ex",
                                name=f"ex{c}_{dt}")
                for qi in range(4):
                    nc.tensor.matmul(
                        ex[:, qi], obdT_sb[:, dt],
                        attn[:, qi].rearrange("p w k -> p k w"),
                        start=True, stop=True)
                exb = av_pool.tile([128, 4, 4, WC], BF16, tag="exb")
                nc.scalar.copy(exb, ex)
                v3 = S[c]["v"][:, dt].rearrange("p (k w) -> p k w", k=4)
                prod = av_pool.tile([128, 4, 4, WC], BF16, tag="prod")
                nc.vector.tensor_mul(
                    prod, exb,
                    v3.unsqueeze(1).to_broadcast([128, 4, 4, WC]))
                # reduce over ki as a packed add tree (cheaper than TensorReduce)
                nc.vector.tensor_add(prod[:, :, 0], prod[:, :, 0], prod[:, :, 1])
                nc.vector.tensor_add(prod[:, :, 2], prod[:, :, 2], prod[:, :, 3])
                nc.vector.tensor_add(
                    o_bf[:, dt].rearrange("p (a w) -> p a w", a=4),
                    prod[:, :, 0], prod[:, :, 2])

            def emit_outproj(c):
                o_bf, xb = S[c]["o_bf"], S[c]["xb"]
                yb = y_pool.tile([128, DT, TC], BF16, tag="y", name=f"y{c}")
                for dto in range(DT):
                    mm = ps_main.tile([128, TC], F32, tag="mm")
                    for dtk in range(DT):
                        nc.tensor.matmul(
                            mm, wo_sb[:, dtk, dto * 128:(dto + 1) * 128],
                            o_bf[:, dtk],
                            start=(dtk == 0), stop=(dtk == DT - 1))
                    t = st_pool.tile([128, TC], BF16, tag="ytmp")
                    nc.scalar.activation(t, mm, AF.Identity,
                                         bias=bo_sb[:, dto:dto + 1])
                    nc.vector.tensor_add(
                        yb[:, dto], t, xb[:, dto].rearrange("p a w -> p (a w)"))
                S[c]["yb"] = yb

            def emit_ln2(c):
                """LN2 stats over 4608 merged features; write h2/res resident."""
                yb = S[c]["yb"]
                w0 = c * WC
                st2x = ps_main.tile([1, TC], F32, tag="mm", name=f"st2x{c}")
                st2q = ps_main.tile([1, TC], F32, tag="mm", name=f"st2q{c}")
                for dt in range(DT):
                    nc.tensor.matmul(st2x, ones_col, yb[:, dt],
                                     start=(dt == 0), stop=(dt == DT - 1))
                for dt in range(DT):
                    ysq = sq_pool.tile([128, TC], BF16, tag="xsq")
                    nc.vector.tensor_mul(ysq, yb[:, dt], yb[:, dt])
                    nc.tensor.matmul(st2q, ones_col, ysq,
                                     start=(dt == 0), stop=(dt == DT - 1))

                # fold the 4 a-positions: [1, (a w)] -> [1, w]
                s2b = st_pool.tile([1, WC], F32, tag="s2b")
                mu2 = st_pool.tile([1, WC], F32, tag="mu2")
                nc.vector.tensor_reduce(
                    mu2, st2x.rearrange("p (a w) -> p w a", a=4),
                    axis=mybir.AxisListType.X, op=ALU.add)
                nc.vector.tensor_scalar_mul(mu2, mu2, 1.0 / D4)
                nc.vector.tensor_reduce(
                    s2b, st2q.rearrange("p (a w) -> p w a", a=4),
                    axis=mybir.AxisListType.X, op=ALU.add)
                var2 = st_pool.tile([1, WC], F32, tag="var2")
                nc.vector.scalar_tensor_tensor(var2, mu2, -1.0, mu2,
                                               ALU.mult, ALU.mult)
                nc.vector.scalar_tensor_tensor(var2, s2b, 1.0 / D4, var2,
                                               ALU.mult, ALU.add)
                rstd2 = st_pool.tile([1, WC], F32, tag="rstd2")
                nc.scalar.activation(rstd2, var2, AF.Sqrt, bias=eps_t)
                nc.vector.reciprocal(rstd2, rstd2)
                nmu2 = mu2  # in-place: mu2 := -mu2 * rstd2
                nc.vector.scalar_tensor_tensor(nmu2, mu2, -1.0, rstd2,
                                               ALU.mult, ALU.mult)

                rstd2_b = bcast_bf16(rstd2, [128, WC], "rstd2")
                nmu2_b = bcast_bf16(nmu2, [128, WC], "nmu2")

                # h2 = (y - mu2) * rstd2 (bf16); res = sum_a(y) (x0.25 in stage B)
                for dt in range(DT):
                    tmp2 = st_pool.tile([128, TC], BF16, tag="normtmp")
                    t2v = tmp2.rearrange("p (a w) -> p a w", a=4)
                    nc.vector.tensor_mul(
                        t2v, yb[:, dt].rearrange("p (a w) -> p a w", a=4),
                        rstd2_b.unsqueeze(1).to_broadcast([128, 4, WC]))
                    nc.vector.tensor_add(
                        h2[:, dt, :, w0:w0 + WC], t2v,
                        nmu2_b.unsqueeze(1).to_broadcast([128, 4, WC]))
                yv = yb.rearrange("p t (a w) -> p t a w", a=4)
                res_sl = res[:, :, w0:w0 + WC]
                nc.vector.tensor_add(res_sl, yv[:, :, 0], yv[:, :, 1])
                nc.vector.tensor_add(res_sl, res_sl, yv[:, :, 2])
                nc.vector.tensor_add(res_sl, res_sl, yv[:, :, 3])

            # ---- pipelined emission -------------------------------------
            # PE order per iteration c:
            #   [qkv(c) interleaved with av(c-1)] outproj(c-1) ln2(c-1)
            #   stats(c+1) scores(c)
            # so the DVE/ACT attention tail of c-1 hides under qkv(c)'s
            # matmuls and PE never waits on softmax/AV chains.
            emit_dma_x(0)
            emit_stats(0)
            emit_finish_norm(0)
            # x(0) is queued; now stream in the weights behind it
            for jt in range(JQ):
                nc.sync.dma_start(wq_sb[:, jt], wqkv_d[:, jt])
            nc.sync.dma_start(wo_sb, wo_d)
            nc.sync.dma_start(b1_sb, b1_d)
            nc.sync.dma_start(ident_sb, ident_d)
            nc.sync.dma_start(w1f_sb, w1_d[0])
            for c in range(CH):
                if c + 1 < CH:
                    emit_dma_x(c + 1)
                for dt in range(DT):
                    emit_qkv_group(c, 3 * dt)
                    if c > 0:
                        emit_av_dt(c - 1, dt)
                    emit_qkv_group(c, 3 * dt + 1)
                    emit_qkv_group(c, 3 * dt + 2)
                if c > 0:
                    emit_outproj(c - 1)
                    emit_ln2(c - 1)
                if c + 1 < CH:
                    emit_stats(c + 1)
                    emit_finish_norm(c + 1)
                emit_ptscores(c)
                emit_softmax(c)
            for dt in range(DT):
                emit_av_dt(CH - 1, dt)
            emit_outproj(CH - 1)
            emit_ln2(CH - 1)

        # =================== Stage B: merger MLP ==========================
        with ExitStack() as bctx:
            acc_pool = bctx.enter_context(tc.tile_pool(name="acc", bufs=1))
            w1_pool = bctx.enter_context(tc.tile_pool(name="w1s", bufs=4))
            m2_pool = bctx.enter_context(tc.tile_pool(name="m2", bufs=2))
            w2_pool = bctx.enter_context(tc.tile_pool(name="w2s", bufs=3))
            fin_pool = bctx.enter_context(tc.tile_pool(name="fin", bufs=1))

            acc = acc_pool.tile([128, DT, NW], F32)
            fin = fin_pool.tile([128, 4, DT, 128], F32)

            for blk in range(NBLK):
                m2 = m2_pool.tile([128, JBLK, NW], BF16)
                for j in range(JBLK):
                    jt = blk * JBLK + j
                    if jt == 0:
                        w1s = w1f_sb
                    else:
                        w1s = w1_pool.tile([128, KT1, 128], BF16)
                        nc.sync.dma_start(w1s, w1_d[jt])
                    mm = ps_main.tile([128, NW], F32, tag="mm")
                    for kt in range(KT1):
                        a, dt = divmod(kt, DT)
                        nc.tensor.matmul(mm, w1s[:, kt], h2[:, dt, a],
                                         start=(kt == 0), stop=(kt == KT1 - 1))
                    nc.scalar.activation(m2[:, j], mm, AF.Gelu_apprx_tanh,
                                         bias=b1_sb[:, jt:jt + 1])

                for dt in range(DT):
                    w2s = w2_pool.tile([128, JBLK, 128], BF16)
                    nc.sync.dma_start(
                        w2s, w2_d[dt, :, blk * JBLK:(blk + 1) * JBLK])
                    mm = ps_main.tile([128, NW], F32, tag="mm")
                    for j in range(JBLK):
                        nc.tensor.matmul(mm, w2s[:, j], m2[:, j],
                                         start=(j == 0), stop=(j == JBLK - 1))
                    if blk == 0:
                        # res holds sum_a(y); x0.25 folds the mean-pool here
                        nc.vector.scalar_tensor_tensor(
                            acc[:, dt], res[:, dt], 0.25, mm, ALU.mult, ALU.add)
                    elif blk == NBLK - 1:
                        nc.vector.scalar_tensor_tensor(
                            acc[:, dt], mm, b2_sb[:, dt:dt + 1], acc[:, dt],
                            ALU.add, ALU.add)
                        # acc[:, dt] final: transpose to token-major and
                        # store, overlapping the DMA with remaining blocks
                        for mt in range(4):
                            tp = ps_main.tile([128, 128], F32, tag="mm")
                            nc.tensor.transpose(
                                tp, acc[:, dt, mt * 128:(mt + 1) * 128],
                                ident_sb)
                            nc.vector.tensor_copy(fin[:, mt, dt], tp)
                        nc.sync.dma_start(
                            out_d.rearrange(
                                "(mt p) (dt q) -> p mt dt q",
                                p=128, q=128)[:, :, dt], fin[:, :, dt])
                    else:
                        nc.vector.tensor_add(acc[:, dt], mm, acc[:, dt])

    nc.compile()
    return nc


# ---------------------------------------------------------------------------
# Host side
# ---------------------------------------------------------------------------

_CACHED = {}


def make_runner(nc, chain=1):
    """Build a reusable jitted SPMD executor for the finalized program.

    Mirrors concourse.bass2jax.run_bass_via_pjrt but caches the jitted
    callable so repeated kernel() calls (and benchmarking) don't recompile.
    With chain=M the NEFF executes M times back-to-back inside one jit call
    (each call consumes the previous call's output buffer, so the chain
    cannot be elided) — used for slope-based timing.
    Returns run(in_maps) -> list[dict] per core.
    """
    import jax
    from jax.sharding import Mesh, PartitionSpec
    from jax.experimental.shard_map import shard_map
    from concourse import mybir as _mybir
    from concourse.bass2jax import (
        install_neuronx_cc_hook, partition_id_tensor, _bass_exec_p)

    install_neuronx_cc_hook()
    partition_name = nc.partition_id_tensor.name if nc.partition_id_tensor else None

    in_names, out_names, out_avals, zero_shapes = [], [], [], []
    for alloc in nc.m.functions[0].allocations:
        if not isinstance(alloc, _mybir.MemoryLocationSet):
            continue
        name = alloc.memorylocations[0].name
        if alloc.kind == "ExternalInput":
            if name != partition_name:
                in_names.append(name)
        elif alloc.kind == "ExternalOutput":
            out_names.append(name)
            shape = tuple(alloc.tensor_shape)
            dtype = _mybir.dt.np(alloc.dtype)
            out_avals.append(jax.core.ShapedArray(shape, dtype))
            zero_shapes.append((shape, dtype))

    n_params = len(in_names)
    n_outs = len(out_avals)
    all_in_names = list(in_names) + list(out_names)
    if partition_name is not None:
        all_in_names.append(partition_name)
    donate = tuple(range(n_params, n_params + n_outs))

    def _body(*args):
        params = list(args[:n_params])
        outs = list(args[n_params:])
        pid = [partition_id_tensor()] if partition_name is not None else []
        for _ in range(chain):
            outs = list(_bass_exec_p.bind(
                *(params + outs + pid),
                out_avals=tuple(out_avals),
                in_names=tuple(all_in_names),
                out_names=tuple(out_names),
                lowering_input_output_aliases=(),
                sim_require_finite=True,
                sim_require_nnan=True,
                nc=nc,
            ))
        return tuple(outs)

    import os
    if os.environ.get("BASS_SIM_CPU") == "1":
        devices = jax.devices("cpu")[:NCORES]
    else:
        devices = jax.devices()[:NCORES]
    mesh = Mesh(np.asarray(devices), ("core",))
    in_specs = (PartitionSpec("core"),) * (n_params + n_outs)
    out_specs = (PartitionSpec("core"),) * n_outs
    sharded = jax.jit(
        shard_map(_body, mesh=mesh, in_specs=in_specs, out_specs=out_specs,
                  check_rep=False),
        donate_argnums=donate, keep_unused=True)

    def make_zeros():
        return [np.zeros((NCORES * s[0], *s[1:]), d) for s, d in zero_shapes]

    def concat_inputs(in_maps):
        return [np.concatenate([np.asarray(in_maps[c][n]) for c in range(NCORES)],
                               axis=0)
                for n in in_names]

    def run(in_maps):
        out_arrs = sharded(*concat_inputs(in_maps), *make_zeros())
        return [
            {n: np.asarray(out_arrs[i]).reshape(NCORES, *out_avals[i].shape)[c]
             for i, n in enumerate(out_names)}
            for c in range(NCORES)
        ]

    run.sharded = sharded
    run.concat_inputs = concat_inputs
    run.make_zeros = make_zeros
    run.out_names = out_names
    run.out_avals = out_avals
    return run


def _prep_weights(ln1_g, ln1_b, w_qkv, b_qkv, w_o, b_o, pre_g, pre_b, w1, b1, w2, b2):
    bf = ml_dtypes.bfloat16
    f32 = np.float32

    ln1_g = np.asarray(ln1_g, f32)
    ln1_b = np.asarray(ln1_b, f32)
    w_qkv = np.asarray(w_qkv, f32)
    w1 = np.asarray(w1, f32)
    w2 = np.asarray(w2, f32)
    w_o = np.asarray(w_o, f32)
    pre_g = np.asarray(pre_g, f32)
    pre_b = np.asarray(pre_b, f32)

    wq = w_qkv * ln1_g[None, :]
    bq = w_qkv @ ln1_b + np.asarray(b_qkv, f32)
    # [p, jt, dt, q] = wq[jt*128+q, dt*128+p]
    wqkv_t = np.ascontiguousarray(
        wq.T.reshape(DT, 128, JQ, 128).transpose(1, 2, 0, 3)).astype(bf)
    bqkv_h = np.ascontiguousarray(bq.reshape(JQ, 128).T)

    # [p, dtk, j] = w_o[j, dtk*128+p]
    wo_t = np.ascontiguousarray(
        w_o.T.reshape(DT, 128, D).transpose(1, 0, 2)).astype(bf)
    bo_h = np.ascontiguousarray(np.asarray(b_o, f32).reshape(DT, 128).T)

    w1g = w1 * pre_g[None, :]
    b1e = w1 @ pre_b + np.asarray(b1, f32)
    w1p = np.zeros((J1P, D4), f32)
    w1p[:J1] = w1g
    # [jt, p, kt, q] = w1p[jt*128+q, kt*128+p]
    w1_t = np.ascontiguousarray(
        w1p.T.reshape(KT1, 128, JT1, 128).transpose(2, 1, 0, 3)).astype(bf)
    b1p = np.zeros((J1P,), f32)
    b1p[:J1] = b1e
    b1_h = np.ascontiguousarray(b1p.reshape(JT1, 128).T)

    w2p = np.zeros((J1P, D), f32)
    w2p[:J1] = w2.T
    # [dt, p, jt, q] = w2p[jt*128+p, dt*128+q]
    w2_t = np.ascontiguousarray(
        w2p.reshape(JT1, 128, DT, 128).transpose(2, 1, 0, 3)).astype(bf)
    b2_h = np.ascontiguousarray(np.asarray(b2, f32).reshape(DT, 128).T)

    heads = (np.arange(D) // HD)
    obd = (heads[:, None] == np.arange(NH)[None, :]).astype(bf)      # [D, NH]
    obd_h = np.ascontiguousarray(obd.reshape(DT, 128, NH).transpose(1, 0, 2))
    obdT_h = np.ascontiguousarray(obd.T.reshape(NH, DT, 128))

    ident_h = np.eye(128, dtype=f32)

    return dict(
        wqkv=wqkv_t, bqkv=bqkv_h, wo=wo_t, bo=bo_h,
        ones_bd=obd_h, ones_bdT=obdT_h,
        w1t=w1_t, b1=b1_h, w2t=w2_t, b2=b2_h, ident=ident_h,
    )


def _shard_x(hidden_states):
    """Full x [1, T, D] -> per-core feature-major bf16 [DT, 128, TS] (a, w)."""
    x = np.asarray(hidden_states, np.float32)[0]          # [T, D]
    nh, nw = H // 2, W // 2
    xr = x.reshape(B, nh, 2, nw, 2, D)
    shards = []
    for c in range(NCORES):
        img, half = divmod(c, 2)
        sl = xr[img, half * 16:(half + 1) * 16]           # [16, 2, 32, 2, D]
        # (a=(r,cc), w=(i,j)) ordering
        sl = sl.transpose(1, 3, 0, 2, 4).reshape(TS, D)   # [(r c i j), D]
        xT = np.ascontiguousarray(sl.T).reshape(DT, 128, TS)
        shards.append(xT.astype(ml_dtypes.bfloat16))
    return shards


def get_runner():
    if "runner" not in _CACHED:
        nc = build_program()
        _CACHED["nc"] = nc
        _CACHED["runner"] = make_runner(nc)
    return _CACHED["runner"]


def make_in_maps(inputs):
    weights = _prep_weights(
        inputs["ln1_g"], inputs["ln1_b"], inputs["w_qkv"], inputs["b_qkv"],
        inputs["w_o"], inputs["b_o"], inputs["pre_g"], inputs["pre_b"],
        inputs["w1"], inputs["b1"], inputs["w2"], inputs["b2"])
    shards = _shard_x(inputs["hidden_states"])
    return [dict(weights, xT=shards[c]) for c in range(NCORES)]


def kernel(**inputs):
    run = get_runner()
    results = run(make_in_maps(inputs))
    out = np.concatenate([results[c]["out"] for c in range(NCORES)], axis=0)
    return out[None].astype(np.float32)


# revision 47
# speedup vs baseline: 1.1560x; 1.0250x over previous
"""Trainium2 Bass kernel for MiniCPMV ViT window-attention + 2x2 merger block.

Architecture (per reference):
  x[1,16384,1152] -> LN1 -> 2x2-window reorder -> QKV -> 4-token window attn
  (16 heads x 72) -> out-proj -> un-reorder + residual -> re-reorder ->
  [4096 windows x 4608] -> LN2 -> Linear(4608->17216) -> gelu(tanh) ->
  Linear(17216->1152) -> + mean-pool residual -> [1,4096,1152]

Key observation: the un-reorder after attention and the re-reorder before the
merger cancel, so everything stays in window order end-to-end and the output
is already in window (= merged token) order.

Sharding: pure data parallel over 8 cores; each core takes 2048 tokens
(512 windows, half of one image = 16 window-rows), weights replicated.
Token order within a core is (a, w): a = position-in-window (0..3),
w = window index (0..511), so per-a slices are contiguous.

v2 scheduling notes:
  - x is shipped bf16; QKV weights live resident in SBUF (loaded once).
  - Stage A is software-pipelined at emission level: PE order per chunk is
    qkv(c), ln1-stats(c+1), scores(c), AV(c), outproj(c), ln2-stats(c) so
    the DVE/ACT phases of chunk c overlap PE matmuls, and normalize of
    chunk c+1 happens during chunk c's attention tail.
  - h2 (LN2-normalized merger input) and res (mean-pool residual) stay
    resident in SBUF; no DRAM round trip between stages.
  - All weights are pre-transposed on the host so every DMA is contiguous.
"""

import numpy as np
import ml_dtypes

import concourse.bacc as bacc
import concourse.tile as tile
import concourse.bass as bass
from concourse import mybir
from concourse.bass_utils import run_bass_kernel_spmd

F32 = mybir.dt.float32
BF16 = mybir.dt.bfloat16
AF = mybir.ActivationFunctionType
ALU = mybir.AluOpType

# Problem constants (hardcoded per spec)
B, H, W, D, I, NH = 4, 64, 64, 1152, 4304, 16
T = B * H * W          # 16384 tokens
HD = D // NH           # 72 head dim
NCORES = 8
TS = T // NCORES       # 2048 tokens per core
NW = TS // 4           # 512 windows per core
DT = D // 128          # 9 feature tiles
JQ = 3 * D // 128      # 27 qkv output tiles
D4 = 4 * D             # 4608 merged feature dim
KT1 = D4 // 128        # 36 contraction tiles for w1
J1 = 4 * I             # 17216
J1P = 17280            # padded to 135*128
JT1 = J1P // 128       # 135
JBLK = 15              # w1 j-tiles per block
NBLK = JT1 // JBLK     # 9 blocks
CH = 8                 # stage-A chunks
WC = NW // CH          # 64 windows per chunk
TC = 4 * WC            # 256 tokens per chunk
EPS = 1e-6
SM_SCALE = 1.0 / np.sqrt(HD)


def build_program():
    """Build the single-core SPMD program (same NEFF on all 8 cores)."""
    from contextlib import ExitStack

    nc = bacc.Bacc("TRN2", target_bir_lowering=False, num_devices=NCORES)

    # ---- DRAM parameters (all laid out for contiguous DMA) ---------------
    def inp(name, shape, dtype):
        return nc.dram_tensor(name, shape, dtype, kind="ExternalInput").ap()

    xT_d = inp("xT", [CH, DT, 128, 4, WC], BF16)     # chunk-major x, contiguous loads
    wqkv_d = inp("wqkv", [128, JQ, DT, 128], BF16)   # [p][jt][dt][q]
    bqkv_d = inp("bqkv", [128, JQ], F32)
    wo_d = inp("wo", [128, DT, D], BF16)             # [p][dtk][j]
    bo_d = inp("bo", [128, DT], F32)
    obd_d = inp("ones_bd", [128, DT, NH], BF16)      # block-diag head masks
    obdT_d = inp("ones_bdT", [NH, DT, 128], BF16)    # head -> feature expand masks
    w1_d = inp("w1t", [JT1, 128, KT1, 128], BF16)    # [jt][p][kt][q]
    b1_d = inp("b1", [128, JT1], F32)
    w2_d = inp("w2t", [DT, 128, JT1, 128], BF16)     # [dt][p][jt][q]
    b2_d = inp("b2", [128, DT], F32)
    ident_d = inp("ident", [128, 128], F32)

    out_d = nc.dram_tensor("out", [NW, D], F32, kind="ExternalOutput").ap()

    with tile.TileContext(nc) as tc, ExitStack() as ctx:
        # ---- pools -------------------------------------------------------
        consts = ctx.enter_context(tc.tile_pool(name="consts", bufs=1))
        ps_main = ctx.enter_context(tc.tile_pool(name="ps_main", bufs=2, space="PSUM"))
        ps_sc = ctx.enter_context(tc.tile_pool(name="ps_sc", bufs=1, space="PSUM"))
        ps_ex = ctx.enter_context(tc.tile_pool(name="ps_ex", bufs=1, space="PSUM"))
        # h2 / res live across both stages
        h2_pool = ctx.enter_context(tc.tile_pool(name="h2f", bufs=1))
        res_pool = ctx.enter_context(tc.tile_pool(name="resf", bufs=1))
        # first w1 tile prefetched during stage A so stage B starts instantly
        w1f_pool = ctx.enter_context(tc.tile_pool(name="w1f", bufs=1))

        # ---- constants (small ones DMA'd now; big ones after x chunk 0) --
        wo_sb = consts.tile([128, DT, D], BF16)
        bqkv_sb = consts.tile([128, JQ], F32)
        nc.sync.dma_start(bqkv_sb, bqkv_d)
        bo_sb = consts.tile([128, DT], F32)
        nc.sync.dma_start(bo_sb, bo_d)
        b1_sb = consts.tile([128, JT1], F32)
        b2_sb = consts.tile([128, DT], F32)
        nc.sync.dma_start(b2_sb, b2_d)
        obd_sb = consts.tile([128, DT, NH], BF16)
        nc.sync.dma_start(obd_sb, obd_d)
        obdT_sb = consts.tile([NH, DT, 128], BF16)
        nc.sync.dma_start(obdT_sb, obdT_d)
        ident_sb = consts.tile([128, 128], F32)

        ones_col = consts.tile([128, 1], BF16)
        nc.vector.memset(ones_col, 1.0)
        eps_t = consts.tile([1, 1], F32)
        nc.vector.memset(eps_t, EPS)

        h2 = h2_pool.tile([128, DT, 4, NW], BF16)    # LN2-normalized, resident
        res = res_pool.tile([128, DT, NW], BF16)     # mean-pool residual
        w1f_sb = w1f_pool.tile([128, KT1, 128], BF16)

        # =================== Stage A: LN1 + attention =====================
        with ExitStack() as actx:
            wq_pool = actx.enter_context(tc.tile_pool(name="wq", bufs=1))
            xb_pool = actx.enter_context(tc.tile_pool(name="xb", bufs=3))
            st_pool = actx.enter_context(tc.tile_pool(name="st", bufs=1))
            sq_pool = actx.enter_context(tc.tile_pool(name="sq", bufs=2))
            h_pool = actx.enter_context(tc.tile_pool(name="h", bufs=2))
            qkv_pool = actx.enter_context(tc.tile_pool(name="qkv", bufs=1))
            v_pool = actx.enter_context(tc.tile_pool(name="v", bufs=2))
            p_pool = actx.enter_context(tc.tile_pool(name="p", bufs=2))
            sm_pool = actx.enter_context(tc.tile_pool(name="sm", bufs=1))
            av_pool = actx.enter_context(tc.tile_pool(name="av", bufs=1))
            o_pool = actx.enter_context(tc.tile_pool(name="o", bufs=1))
            y_pool = actx.enter_context(tc.tile_pool(name="y", bufs=1))

            wq_sb = wq_pool.tile([128, JQ, DT, 128], BF16)

            S = [dict() for _ in range(CH)]

            def emit_dma_x(c):
                xb = xb_pool.tile([128, DT, 4, WC], BF16, tag="xb", name=f"xb{c}")
                for dt in range(DT):
                    nc.sync.dma_start(xb[:, dt], xT_d[c, dt])
                S[c]["xb"] = xb

            def emit_stats(c):
                """LN1 stats: col sums of x and x^2 via ones-vector matmul."""
                xb = S[c]["xb"]
                stx = ps_main.tile([1, TC], F32, tag="mm", name=f"stx{c}")
                stq = ps_main.tile([1, TC], F32, tag="mm", name=f"stq{c}")
                for dt in range(DT):
                    nc.tensor.matmul(stx,
                                     ones_col,
                                     xb[:, dt].rearrange("p a w -> p (a w)"),
                                     start=(dt == 0), stop=(dt == DT - 1))
                for dt in range(DT):
                    xsq = sq_pool.tile([128, TC], BF16, tag="xsq")
                    nc.vector.tensor_mul(
                        xsq, xb[:, dt].rearrange("p a w -> p (a w)"),
                        xb[:, dt].rearrange("p a w -> p (a w)"))
                    nc.tensor.matmul(stq, ones_col, xsq,
                                     start=(dt == 0), stop=(dt == DT - 1))
                S[c]["stx"], S[c]["stq"] = stx, stq

            def bcast_bf16(val_f32, shape, tagp):
                """f32 [1, n] stats -> bf16 [128, n] broadcast tile."""
                n = shape[1]
                vb = st_pool.tile([1, n], BF16, tag=f"{tagp}_v")
                with nc.allow_low_precision(reason="LN scale bf16"):
                    nc.vector.tensor_copy(vb, val_f32)
                out = st_pool.tile([128, n], BF16, tag=f"{tagp}_b")
                nc.gpsimd.partition_broadcast(out, vb)
                return out

            def emit_finish_norm(c):
                xb = S[c]["xb"]
                mu = st_pool.tile([1, TC], F32, tag="mu")
                nc.vector.tensor_scalar_mul(mu, S[c]["stx"], 1.0 / D)
                var = st_pool.tile([1, TC], F32, tag="var")
                nc.vector.scalar_tensor_tensor(var, mu, -1.0, mu, ALU.mult, ALU.mult)
                nc.vector.scalar_tensor_tensor(var, S[c]["stq"], 1.0 / D, var,
                                               ALU.mult, ALU.add)
                rstd = st_pool.tile([1, TC], F32, tag="rstd")
                nc.scalar.activation(rstd, var, AF.Sqrt, bias=eps_t)
                nc.vector.reciprocal(rstd, rstd)
                nmu = mu  # in-place: mu := -mu * rstd
                nc.vector.scalar_tensor_tensor(nmu, mu, -1.0, rstd, ALU.mult, ALU.mult)

                rstd_b = bcast_bf16(rstd, [128, TC], "rstd")
                nmu_b = bcast_bf16(nmu, [128, TC], "nmu")

                h = h_pool.tile([128, DT, TC], BF16, tag="h", name=f"h{c}")
                for dt in range(DT):
                    tmp = st_pool.tile([128, TC], BF16, tag="normtmp")
                    nc.vector.tensor_mul(
                        tmp, xb[:, dt].rearrange("p a w -> p (a w)"), rstd_b)
                    nc.vector.tensor_add(h[:, dt], tmp, nmu_b)
                S[c]["h"] = h

            def emit_qkv_group(c, jt):
                h = S[c]["h"]
                if jt == 0:
                    S[c]["qk"] = qkv_pool.tile([128, 2 * DT, TC], BF16,
                                               tag="qk", name=f"qk{c}")
                    S[c]["v"] = v_pool.tile([128, DT, TC], BF16,
                                            tag="v", name=f"v{c}")
                mm = ps_main.tile([128, TC], F32, tag="mm")
                for dt in range(DT):
                    nc.tensor.matmul(mm, wq_sb[:, jt, dt], h[:, dt],
                                     start=(dt == 0), stop=(dt == DT - 1))
                dst = (S[c]["qk"][:, jt] if jt < 2 * DT
                       else S[c]["v"][:, jt - 2 * DT])
                nc.scalar.activation(dst, mm, AF.Identity,
                                     bias=bqkv_sb[:, jt:jt + 1])

            def emit_ptscores(c):
                qkv = S[c]["qk"]
                scs = [ps_sc.tile([NH, 4, WC], F32, tag=f"sc{qi}",
                                  name=f"sc{qi}_{c}") for qi in range(4)]
                for dt in range(DT):
                    q3 = qkv[:, dt].rearrange("p (a w) -> p a w", a=4)
                    k3 = qkv[:, DT + dt].rearrange("p (a w) -> p a w", a=4)
                    p_t = p_pool.tile([128, 4, 4, WC], BF16)
                    nc.vector.tensor_mul(
                        p_t,
                        q3.unsqueeze(2).to_broadcast([128, 4, 4, WC]),
                        k3.unsqueeze(1).to_broadcast([128, 4, 4, WC]),
                    )
                    for qi in range(4):
                        nc.tensor.matmul(
                            scs[qi], obd_sb[:, dt],
                            p_t[:, qi].rearrange("p a w -> p (a w)"),
                            start=(dt == 0), stop=(dt == DT - 1))
                S[c]["scs"] = scs

            def emit_softmax(c):
                scs = S[c]["scs"]
                # esb: [h, qi, w, ki] bf16, ki innermost
                esb = sm_pool.tile([NH, 4, WC, 4], BF16, tag="esb")
                for qi in range(4):
                    nc.scalar.activation(
                        esb[:, qi].rearrange("p w k -> p k w"), scs[qi],
                        AF.Exp, scale=float(SM_SCALE))
                den = sm_pool.tile([NH, 4, WC], BF16, tag="den")
                with nc.allow_low_precision(reason="softmax denom bf16"):
                    nc.vector.tensor_reduce(den, esb, axis=mybir.AxisListType.X,
                                            op=ALU.add)
                    nc.vector.reciprocal(den, den)
                attn = esb  # in-place normalize
                nc.vector.tensor_mul(
                    attn, esb, den.unsqueeze(3).to_broadcast([NH, 4, WC, 4]))
                S[c]["attn"] = attn

            def emit_av_dt(c, dt):
                attn = S[c]["attn"]
                if dt == 0:
                    S[c]["o_bf"] = o_pool.tile([128, DT, 4 * WC], BF16, tag="o",
                                               name=f"o{c}")
                o_bf = S[c]["o_bf"]
                # expand: ex[p, qi, ki, w] = attn[h(p), qi, w, ki]
                ex = ps_ex.tile([128, 4, 4, WC], F32, tag="ex",
                                name=f"ex{c}_{dt}")
                for qi in range(4):
                    nc.tensor.matmul(
                        ex[:, qi], obdT_sb[:, dt],
                        attn[:, qi].rearrange("p w k -> p k w"),
                        start=True, stop=True)
                exb = av_pool.tile([128, 4, 4, WC], BF16, tag="exb")
                nc.scalar.copy(exb, ex)
                v3 = S[c]["v"][:, dt].rearrange("p (k w) -> p k w", k=4)
                prod = av_pool.tile([128, 4, 4, WC], BF16, tag="prod")
                nc.vector.tensor_mul(
                    prod, exb,
                    v3.unsqueeze(1).to_broadcast([128, 4, 4, WC]))
                # reduce over ki as a packed add tree (cheaper than TensorReduce)
                nc.vector.tensor_add(prod[:, :, 0], prod[:, :, 0], prod[:, :, 1])
                nc.vector.tensor_add(prod[:, :, 2], prod[:, :, 2], prod[:, :, 3])
                nc.vector.tensor_add(
                    o_bf[:, dt].rearrange("p (a w) -> p a w", a=4),
                    prod[:, :, 0], prod[:, :, 2])

            def emit_outproj(c):
                o_bf, xb = S[c]["o_bf"], S[c]["xb"]
                yb = y_pool.tile([128, DT, TC], BF16, tag="y", name=f"y{c}")
                for dto in range(DT):
                    mm = ps_main.tile([128, TC], F32, tag="mm")
                    for dtk in range(DT):
                        nc.tensor.matmul(
                            mm, wo_sb[:, dtk, dto * 128:(dto + 1) * 128],
                            o_bf[:, dtk],
                            start=(dtk == 0), stop=(dtk == DT - 1))
                    t = st_pool.tile([128, TC], BF16, tag="ytmp")
                    nc.scalar.activation(t, mm, AF.Identity,
                                         bias=bo_sb[:, dto:dto + 1])
                    nc.vector.tensor_add(
                        yb[:, dto], t, xb[:, dto].rearrange("p a w -> p (a w)"))
                S[c]["yb"] = yb

            def emit_ln2(c):
                """LN2 stats over 4608 merged features; write h2/res resident."""
                yb = S[c]["yb"]
                w0 = c * WC
                st2x = ps_main.tile([1, TC], F32, tag="mm", name=f"st2x{c}")
                st2q = ps_main.tile([1, TC], F32, tag="mm", name=f"st2q{c}")
                for dt in range(DT):
                    nc.tensor.matmul(st2x, ones_col, yb[:, dt],
                                     start=(dt == 0), stop=(dt == DT - 1))
                for dt in range(DT):
                    ysq = sq_pool.tile([128, TC], BF16, tag="xsq")
                    nc.vector.tensor_mul(ysq, yb[:, dt], yb[:, dt])
                    nc.tensor.matmul(st2q, ones_col, ysq,
                                     start=(dt == 0), stop=(dt == DT - 1))

                # fold the 4 a-positions: [1, (a w)] -> [1, w]
                s2b = st_pool.tile([1, WC], F32, tag="s2b")
                mu2 = st_pool.tile([1, WC], F32, tag="mu2")
                nc.vector.tensor_reduce(
                    mu2, st2x.rearrange("p (a w) -> p w a", a=4),
                    axis=mybir.AxisListType.X, op=ALU.add)
                nc.vector.tensor_scalar_mul(mu2, mu2, 1.0 / D4)
                nc.vector.tensor_reduce(
                    s2b, st2q.rearrange("p (a w) -> p w a", a=4),
                    axis=mybir.AxisListType.X, op=ALU.add)
                var2 = st_pool.tile([1, WC], F32, tag="var2")
                nc.vector.scalar_tensor_tensor(var2, mu2, -1.0, mu2,
                                               ALU.mult, ALU.mult)
                nc.vector.scalar_tensor_tensor(var2, s2b, 1.0 / D4, var2,
                                               ALU.mult, ALU.add)
                rstd2 = st_pool.tile([1, WC], F32, tag="rstd2")
                nc.scalar.activation(rstd2, var2, AF.Sqrt, bias=eps_t)
                nc.vector.reciprocal(rstd2, rstd2)
                nmu2 = mu2  # in-place: mu2 := -mu2 * rstd2
                nc.vector.scalar_tensor_tensor(nmu2, mu2, -1.0, rstd2,
                                               ALU.mult, ALU.mult)

                rstd2_b = bcast_bf16(rstd2, [128, WC], "rstd2")
                nmu2_b = bcast_bf16(nmu2, [128, WC], "nmu2")

                # h2 = (y - mu2) * rstd2 (bf16); res = sum_a(y) (x0.25 in stage B)
                for dt in range(DT):
                    tmp2 = st_pool.tile([128, TC], BF16, tag="normtmp")
                    t2v = tmp2.rearrange("p (a w) -> p a w", a=4)
                    nc.vector.tensor_mul(
                        t2v, yb[:, dt].rearrange("p (a w) -> p a w", a=4),
                        rstd2_b.unsqueeze(1).to_broadcast([128, 4, WC]))
                    nc.vector.tensor_add(
                        h2[:, dt, :, w0:w0 + WC], t2v,
                        nmu2_b.unsqueeze(1).to_broadcast([128, 4, WC]))
                yv = yb.rearrange("p t (a w) -> p t a w", a=4)
                res_sl = res[:, :, w0:w0 + WC]
                nc.vector.tensor_add(res_sl, yv[:, :, 0], yv[:, :, 1])
                nc.vector.tensor_add(res_sl, res_sl, yv[:, :, 2])
                nc.vector.tensor_add(res_sl, res_sl, yv[:, :, 3])

            # ---- pipelined emission -------------------------------------
            # PE order per iteration c:
            #   [qkv(c) interleaved with av(c-1)] outproj(c-1) ln2(c-1)
            #   stats(c+1) scores(c)
            # so the DVE/ACT attention tail of c-1 hides under qkv(c)'s
            # matmuls and PE never waits on softmax/AV chains.
            emit_dma_x(0)
            emit_stats(0)
            emit_finish_norm(0)
            # x(0) is queued; now stream in the weights behind it
            for jt in range(JQ):
                nc.sync.dma_start(wq_sb[:, jt], wqkv_d[:, jt])
            nc.sync.dma_start(wo_sb, wo_d)
            nc.sync.dma_start(b1_sb, b1_d)
            nc.sync.dma_start(ident_sb, ident_d)
            nc.sync.dma_start(w1f_sb, w1_d[0])
            for c in range(CH):
                if c + 1 < CH:
                    emit_dma_x(c + 1)
                for dt in range(DT):
                    emit_qkv_group(c, 3 * dt)
                    if c > 0:
                        emit_av_dt(c - 1, dt)
                    emit_qkv_group(c, 3 * dt + 1)
                    emit_qkv_group(c, 3 * dt + 2)
                if c > 0:
                    emit_outproj(c - 1)
                    emit_ln2(c - 1)
                if c + 1 < CH:
                    emit_stats(c + 1)
                    emit_finish_norm(c + 1)
                emit_ptscores(c)
                emit_softmax(c)
            for dt in range(DT):
                emit_av_dt(CH - 1, dt)
            emit_outproj(CH - 1)
            emit_ln2(CH - 1)

        # =================== Stage B: merger MLP ==========================
        with ExitStack() as bctx:
            acc_pool = bctx.enter_context(tc.tile_pool(name="acc", bufs=1))
            w1_pool = bctx.enter_context(tc.tile_pool(name="w1s", bufs=4))
            m2_pool = bctx.enter_context(tc.tile_pool(name="m2", bufs=2))
            w2_pool = bctx.enter_context(tc.tile_pool(name="w2s", bufs=3))
            fin_pool = bctx.enter_context(tc.tile_pool(name="fin", bufs=1))

            acc = acc_pool.tile([128, DT, NW], F32)
            fin = fin_pool.tile([128, 4, DT, 128], F32)

            for blk in range(NBLK):
                m2 = m2_pool.tile([128, JBLK, NW], BF16)
                for j in range(JBLK):
                    jt = blk * JBLK + j
                    if jt == 0:
                        w1s = w1f_sb
                    else:
                        w1s = w1_pool.tile([128, KT1, 128], BF16)
                        nc.sync.dma_start(w1s, w1_d[jt])
                    mm = ps_main.tile([128, NW], F32, tag="mm")
                    for kt in range(KT1):
                        a, dt = divmod(kt, DT)
                        nc.tensor.matmul(mm, w1s[:, kt], h2[:, dt, a],
                                         start=(kt == 0), stop=(kt == KT1 - 1))
                    nc.scalar.activation(m2[:, j], mm, AF.Gelu_apprx_tanh,
                                         bias=b1_sb[:, jt:jt + 1])

                for dt in range(DT):
                    w2s = w2_pool.tile([128, JBLK, 128], BF16)
                    nc.sync.dma_start(
                        w2s, w2_d[dt, :, blk * JBLK:(blk + 1) * JBLK])
                    mm = ps_main.tile([128, NW], F32, tag="mm")
                    for j in range(JBLK):
                        nc.tensor.matmul(mm, w2s[:, j], m2[:, j],
                                         start=(j == 0), stop=(j == JBLK - 1))
                    if blk == 0:
                        # res holds sum_a(y); x0.25 folds the mean-pool here
                        nc.vector.scalar_tensor_tensor(
                            acc[:, dt], res[:, dt], 0.25, mm, ALU.mult, ALU.add)
                    elif blk == NBLK - 1:
                        nc.vector.scalar_tensor_tensor(
                            acc[:, dt], mm, b2_sb[:, dt:dt + 1], acc[:, dt],
                            ALU.add, ALU.add)
                        # acc[:, dt] final: transpose to token-major and
                        # store, overlapping the DMA with remaining blocks
                        for mt in range(4):
                            tp = ps_main.tile([128, 128], F32, tag="mm")
                            nc.tensor.transpose(
                                tp, acc[:, dt, mt * 128:(mt + 1) * 128],
                                ident_sb)
                            nc.vector.tensor_copy(fin[:, mt, dt], tp)
                        nc.sync.dma_start(
                            out_d.rearrange(
                                "(mt p) (dt q) -> p mt dt q",
                                p=128, q=128)[:, :, dt], fin[:, :, dt])
                    else:
                        nc.vector.tensor_add(acc[:, dt], mm, acc[:, dt])

    nc.compile()
    return nc


# ---------------------------------------------------------------------------
# Host side
# ---------------------------------------------------------------------------

_CACHED = {}


def make_runner(nc, chain=1):
    """Build a reusable jitted SPMD executor for the finalized program.

    Mirrors concourse.bass2jax.run_bass_via_pjrt but caches the jitted
    callable so repeated kernel() calls (and benchmarking) don't recompile.
    With chain=M the NEFF executes M times back-to-back inside one jit call
    (each call consumes the previous call's output buffer, so the chain
    cannot be elided) — used for slope-based timing.
    Returns run(in_maps) -> list[dict] per core.
    """
    import jax
    from jax.sharding import Mesh, PartitionSpec
    from jax.experimental.shard_map import shard_map
    from concourse import mybir as _mybir
    from concourse.bass2jax import (
        install_neuronx_cc_hook, partition_id_tensor, _bass_exec_p)

    install_neuronx_cc_hook()
    partition_name = nc.partition_id_tensor.name if nc.partition_id_tensor else None

    in_names, out_names, out_avals, zero_shapes = [], [], [], []
    for alloc in nc.m.functions[0].allocations:
        if not isinstance(alloc, _mybir.MemoryLocationSet):
            continue
        name = alloc.memorylocations[0].name
        if alloc.kind == "ExternalInput":
            if name != partition_name:
                in_names.append(name)
        elif alloc.kind == "ExternalOutput":
            out_names.append(name)
            shape = tuple(alloc.tensor_shape)
            dtype = _mybir.dt.np(alloc.dtype)
            out_avals.append(jax.core.ShapedArray(shape, dtype))
            zero_shapes.append((shape, dtype))

    n_params = len(in_names)
    n_outs = len(out_avals)
    all_in_names = list(in_names) + list(out_names)
    if partition_name is not None:
        all_in_names.append(partition_name)
    donate = tuple(range(n_params, n_params + n_outs))

    def _body(*args):
        params = list(args[:n_params])
        outs = list(args[n_params:])
        pid = [partition_id_tensor()] if partition_name is not None else []
        for _ in range(chain):
            outs = list(_bass_exec_p.bind(
                *(params + outs + pid),
                out_avals=tuple(out_avals),
                in_names=tuple(all_in_names),
                out_names=tuple(out_names),
                lowering_input_output_aliases=(),
                sim_require_finite=True,
                sim_require_nnan=True,
                nc=nc,
            ))
        return tuple(outs)

    import os
    if os.environ.get("BASS_SIM_CPU") == "1":
        devices = jax.devices("cpu")[:NCORES]
    else:
        devices = jax.devices()[:NCORES]
    mesh = Mesh(np.asarray(devices), ("core",))
    in_specs = (PartitionSpec("core"),) * (n_params + n_outs)
    out_specs = (PartitionSpec("core"),) * n_outs
    sharded = jax.jit(
        shard_map(_body, mesh=mesh, in_specs=in_specs, out_specs=out_specs,
                  check_rep=False),
        donate_argnums=donate, keep_unused=True)

    def make_zeros():
        return [np.zeros((NCORES * s[0], *s[1:]), d) for s, d in zero_shapes]

    def concat_inputs(in_maps):
        return [np.concatenate([np.asarray(in_maps[c][n]) for c in range(NCORES)],
                               axis=0)
                for n in in_names]

    def run(in_maps):
        out_arrs = sharded(*concat_inputs(in_maps), *make_zeros())
        return [
            {n: np.asarray(out_arrs[i]).reshape(NCORES, *out_avals[i].shape)[c]
             for i, n in enumerate(out_names)}
            for c in range(NCORES)
        ]

    run.sharded = sharded
    run.concat_inputs = concat_inputs
    run.make_zeros = make_zeros
    run.out_names = out_names
    run.out_avals = out_avals
    return run


def _prep_weights(ln1_g, ln1_b, w_qkv, b_qkv, w_o, b_o, pre_g, pre_b, w1, b1, w2, b2):
    bf = ml_dtypes.bfloat16
    f32 = np.float32

    ln1_g = np.asarray(ln1_g, f32)
    ln1_b = np.asarray(ln1_b, f32)
    w_qkv = np.asarray(w_qkv, f32)
    w1 = np.asarray(w1, f32)
    w2 = np.asarray(w2, f32)
    w_o = np.asarray(w_o, f32)
    pre_g = np.asarray(pre_g, f32)
    pre_b = np.asarray(pre_b, f32)

    wq = w_qkv * ln1_g[None, :]
    bq = w_qkv @ ln1_b + np.asarray(b_qkv, f32)
    # [p, jt, dt, q] = wq[jt*128+q, dt*128+p]
    wqkv_t = np.ascontiguousarray(
        wq.T.reshape(DT, 128, JQ, 128).transpose(1, 2, 0, 3)).astype(bf)
    bqkv_h = np.ascontiguousarray(bq.reshape(JQ, 128).T)

    # [p, dtk, j] = w_o[j, dtk*128+p]
    wo_t = np.ascontiguousarray(
        w_o.T.reshape(DT, 128, D).transpose(1, 0, 2)).astype(bf)
    bo_h = np.ascontiguousarray(np.asarray(b_o, f32).reshape(DT, 128).T)

    w1g = w1 * pre_g[None, :]
    b1e = w1 @ pre_b + np.asarray(b1, f32)
    w1p = np.zeros((J1P, D4), f32)
    w1p[:J1] = w1g
    # [jt, p, kt, q] = w1p[jt*128+q, kt*128+p]
    w1_t = np.ascontiguousarray(
        w1p.T.reshape(KT1, 128, JT1, 128).transpose(2, 1, 0, 3)).astype(bf)
    b1p = np.zeros((J1P,), f32)
    b1p[:J1] = b1e
    b1_h = np.ascontiguousarray(b1p.reshape(JT1, 128).T)

    w2p = np.zeros((J1P, D), f32)
    w2p[:J1] = w2.T
    # [dt, p, jt, q] = w2p[jt*128+p, dt*128+q]
    w2_t = np.ascontiguousarray(
        w2p.reshape(JT1, 128, DT, 128).transpose(2, 1, 0, 3)).astype(bf)
    b2_h = np.ascontiguousarray(np.asarray(b2, f32).reshape(DT, 128).T)

    heads = (np.arange(D) // HD)
    obd = (heads[:, None] == np.arange(NH)[None, :]).astype(bf)      # [D, NH]
    obd_h = np.ascontiguousarray(obd.reshape(DT, 128, NH).transpose(1, 0, 2))
    obdT_h = np.ascontiguousarray(obd.T.reshape(NH, DT, 128))

    ident_h = np.eye(128, dtype=f32)

    return dict(
        wqkv=wqkv_t, bqkv=bqkv_h, wo=wo_t, bo=bo_h,
        ones_bd=obd_h, ones_bdT=obdT_h,
        w1t=w1_t, b1=b1_h, w2t=w2_t, b2=b2_h, ident=ident_h,
    )


def _shard_x(hidden_states):
    """Full x [1, T, D] -> per-core feature-major bf16 [DT, 128, TS] (a, w)."""
    x = np.asarray(hidden_states, np.float32)[0]          # [T, D]
    nh, nw = H // 2, W // 2
    xr = x.reshape(B, nh, 2, nw, 2, D)
    shards = []
    for c in range(NCORES):
        img, half = divmod(c, 2)
        sl = xr[img, half * 16:(half + 1) * 16]           # [16, 2, 32, 2, D]
        # (a=(r,cc), w=(i,j)) ordering
        sl = sl.transpose(1, 3, 0, 2, 4).reshape(TS, D)   # [(r c i j), D]
        xT = sl.T.reshape(DT, 128, 4, CH, WC)             # w = (chunk, wc)
        xT = np.ascontiguousarray(xT.transpose(3, 0, 1, 2, 4))
        shards.append(xT.astype(ml_dtypes.bfloat16))
    return shards


def get_runner():
    if "runner" not in _CACHED:
        nc = build_program()
        _CACHED["nc"] = nc
        _CACHED["runner"] = make_runner(nc)
    return _CACHED["runner"]


def make_in_maps(inputs):
    weights = _prep_weights(
        inputs["ln1_g"], inputs["ln1_b"], inputs["w_qkv"], inputs["b_qkv"],
        inputs["w_o"], inputs["b_o"], inputs["pre_g"], inputs["pre_b"],
        inputs["w1"], inputs["b1"], inputs["w2"], inputs["b2"])
    shards = _shard_x(inputs["hidden_states"])
    return [dict(weights, xT=shards[c]) for c in range(NCORES)]


def kernel(**inputs):
    run = get_runner()
    results = run(make_in_maps(inputs))
    out = np.concatenate([results[c]["out"] for c in range(NCORES)], axis=0)
    return out[None].astype(np.float32)
